# revision 1
# baseline (speedup 1.0000x reference)
"""Trainium2 Bass kernel for a GQA sliding-window attention layer.

Reference computation (B=2, T=2048, C=2048, 16 Q heads / 4 KV heads, d=128):
    q = x @ Wq; k = x @ Wk; v = x @ Wv (+ sigmoid-gated value embedding)
    q, k = rmsnorm(rope(q)), rmsnorm(rope(k))
    scores masked to the band 0 <= j - i < window (=1024), softmax over j
    out = (p @ v) @ Wo

Sharding: 8 cores = 2 batches x 4 KV groups.  Each core computes its 4 Q
heads / 1 KV head for one batch and a partial output (its 512-row slice of
the Wo contraction); the host sums the 4 partials per batch.

Layout strategy per core:
  - xT (C x T, bf16) resident in SBUF; all projections contract over C.
  - q̂T / k̂T kept [d=128 partitions, T free]; scores computed transposed
    (S^T tiles [kj, qi]) so that P^T feeds the PV matmul directly with v in
    natural [token, d] layout (no P transposes).
  - softmax has no max-subtraction: rms-normalized q,k bound |score| by
    sqrt(128), so exp is safe in fp32.
  - per-q softmax denominators and rms rows are broadcast across partitions
    via a tiny DRAM bounce (SBUF APs need nonzero partition stride).
"""

import numpy as np
import ml_dtypes
from collections import deque

BF16 = ml_dtypes.bfloat16

# Problem dims (hardcoded per contest rules)
B, T, C = 2, 2048, 2048
N_HEAD, N_KV, HD, GATE_CH = 16, 4, 128, 32
WINDOW = 1024
P = 128
GH = N_HEAD // N_KV  # q heads per kv head (= per core)
N_CORES = 8

_PROGRAM_CACHE = {}


def build_program(T_=T, C_=C, win=WINDOW):
    import concourse.mybir as mybir
    import concourse.tile as tile
    from concourse import bacc

    dt = mybir.dt
    f32 = dt.float32
    bf16 = dt.bfloat16
    AF = mybir.ActivationFunctionType
    ALU = mybir.AluOpType

    NT = T_ // P          # token tiles
    KT = C_ // P          # contraction tiles
    WT = win // P         # window tiles
    ISQ = 1.0 / float(np.sqrt(HD))

    nc = bacc.Bacc()

    xT = nc.declare_dram_parameter("xT", [C_, T_], bf16, isOutput=False)
    wq = nc.declare_dram_parameter("wq", [C_, GH * HD], bf16, isOutput=False)
    wk = nc.declare_dram_parameter("wk", [C_, HD], bf16, isOutput=False)
    wv = nc.declare_dram_parameter("wv", [C_, HD], bf16, isOutput=False)
    wg = nc.declare_dram_parameter("wg", [GATE_CH, 1], bf16, isOutput=False)
    ve2 = nc.declare_dram_parameter("ve2", [T_, HD], bf16, isOutput=False)
    wo = nc.declare_dram_parameter("wo", [GH * HD, C_], bf16, isOutput=False)
    ccd = nc.declare_dram_parameter("cc", [P, T_], bf16, isOutput=False)
    ssd = nc.declare_dram_parameter("ss", [P, T_], bf16, isOutput=False)
    tlo = nc.declare_dram_parameter("tlo", [P, P], bf16, isOutput=False)
    thi = nc.declare_dram_parameter("thi", [P, P], bf16, isOutput=False)
    idr = nc.declare_dram_parameter("identr", [P, GH * P], bf16, isOutput=False)
    idf = nc.declare_dram_parameter("identf", [P, P], f32, isOutput=False)
    out_d = nc.declare_dram_parameter("out", [T_, C_], f32, isOutput=True)
    f32r = dt.float32r

    with tile.TileContext(nc) as tc:
        with (
            tc.tile_pool(name="singles", bufs=1) as sg,
            tc.tile_pool(name="work", bufs=2) as wk_pool,
            tc.tile_pool(name="work3", bufs=4) as w3_pool,
            tc.tile_pool(name="attw", bufs=4) as aw,
            tc.tile_pool(name="yup", bufs=3) as yu_pool,
            tc.tile_pool(name="outp", bufs=3) as op_pool,
            tc.tile_pool(name="psum", bufs=8, space="PSUM") as pp,
        ):
            # ---- persistent inputs -------------------------------------
            # weight/x DMAs are split per k-tile and interleaved so the
            # first projection matmuls (kt=0) can start almost immediately
            # small constants FIRST (the rope tails read cc/ss early — they
            # must not queue behind the 13MB of x/weight traffic)
            wg_sb = sg.tile([GATE_CH, 1], bf16, tag="wg")
            nc.sync.dma_start(out=wg_sb[:], in_=wg[:])
            cc_sb = sg.tile([P, T_], bf16, tag="cc")
            nc.sync.dma_start(out=cc_sb[:], in_=ccd[:])
            ss_sb = sg.tile([P, T_], bf16, tag="ss")
            nc.sync.dma_start(out=ss_sb[:], in_=ssd[:])
            ve2_sb = sg.tile([P, NT, HD], bf16, tag="ve2")
            nc.sync.dma_start(out=ve2_sb[:], in_=ve2.rearrange("(o p) d -> p o d", p=P))
            tlo_sb = sg.tile([P, P], bf16, tag="tlo")
            nc.sync.dma_start(out=tlo_sb[:], in_=tlo[:])
            thi_sb = sg.tile([P, P], bf16, tag="thi")
            nc.sync.dma_start(out=thi_sb[:], in_=thi[:])
            idr_sb = sg.tile([P, GH * P], bf16, tag="idr")
            nc.sync.dma_start(out=idr_sb[:], in_=idr[:])
            idf_sb = sg.tile([P, P], f32, tag="idf")
            nc.sync.dma_start(out=idf_sb[:], in_=idf[:])
            xt = []
            wq_sb = sg.tile([P, KT, GH * HD], bf16, tag="wq")
            wk_sb = sg.tile([P, KT, HD], bf16, tag="wk")
            wv_sb = sg.tile([P, KT, HD], bf16, tag="wv")
            wqr = wq.rearrange("(o p) n -> p o n", p=P)
            wkr = wk.rearrange("(o p) n -> p o n", p=P)
            wvr = wv.rearrange("(o p) n -> p o n", p=P)
            for kt in range(KT):
                t_ = sg.tile([P, T_], bf16, tag=f"xt{kt}")
                nc.sync.dma_start(out=wk_sb[:, kt, :], in_=wkr[:, kt, :])
                nc.sync.dma_start(out=t_[:], in_=xT[kt * P:(kt + 1) * P, :])
                xt.append(t_)
                nc.sync.dma_start(out=wq_sb[:, kt, :], in_=wqr[:, kt, :])
                nc.sync.dma_start(out=wv_sb[:, kt, :], in_=wvr[:, kt, :])
            wo_sb = sg.tile([P, GH, C_], bf16, tag="wo")
            nc.sync.dma_start(out=wo_sb[:], in_=wo.rearrange("(o p) n -> p o n", p=P))
            ones_sb = sg.tile([P, 1], bf16, tag="onesb")
            nc.vector.memset(ones_sb[:], 1.0)
            ones1f = sg.tile([1, P], f32, tag="ones1f")
            nc.vector.memset(ones1f[:], 1.0)
            eps_sb = sg.tile([P, 1], f32, tag="epsb")
            nc.vector.memset(eps_sb[:], 1e-6)

            # persistent intermediates
            qhat = sg.tile([P, GH, T_], bf16, tag="qhat")   # normalized roped q, [d, h, t]
            khat = sg.tile([P, T_], bf16, tag="khat")       # normalized roped k * isq
            vsb = sg.tile([P, NT, HD], bf16, tag="vsb")     # gated v, [tok, tt, d]

            TS = T_ // 512  # 512-wide token slices

            # ---- projections + rope + rmsnorm for k/q heads and vT -----
            # Emitted as kt-major WAVES of 3 output groups: the PE chases the
            # xT DMAs tile-by-tile during the ramp, and each wave's dependent
            # tail work (rope/rms/broadcast) is batched behind the next
            # wave's matmuls so the PE stream never waits on DVE/ACT chains.
            def wave_mms(wave):
                items = []
                for (head, ts_) in wave:
                    sl = slice(ts_ * 512, ts_ * 512 + 512)
                    ps = pp.tile([P, 512], f32, tag="pb",
                                 name=f"ps{head}_{ts_}")
                    items.append((head, sl, ps))
                for kt in range(KT):
                    for gi, (head, ts_) in enumerate(wave):
                        if head == 0:
                            w_ap = wk_sb[:, kt, :]
                        elif head == GH + 1:
                            w_ap = wv_sb[:, kt, :]
                        else:
                            w_ap = wq_sb[:, kt, (head - 1) * HD:head * HD]
                        nc.tensor.matmul(
                            items[gi][2][:], lhsT=w_ap,
                            rhs=xt[kt][:, items[gi][1]],
                            start=(kt == 0), stop=(kt == KT - 1),
                        )
                return items

            def v_tail(head, sl, ps):
                # vT psum [d, tok] -> sbuf f32, then PE-transpose each 128-tok
                # block to natural [tok, d] and add the sigmoid-gated ve.
                vt = wk_pool.tile([P, 512], f32, tag="vt")
                nc.vector.tensor_copy(vt[:], ps[:])
                for i in range(4):
                    tt = sl.start // P + i
                    tsl = slice(tt * P, (tt + 1) * P)
                    tp = pp.tile([P, P], f32, tag="pb")
                    nc.tensor.transpose(tp[:], vt[:, i * P:(i + 1) * P], idf_sb[:])
                    gps = pp.tile([P, 1], f32, tag="pb")
                    nc.tensor.matmul(gps[:], lhsT=xt[0][0:GATE_CH, tsl],
                                     rhs=wg_sb[:], start=True, stop=True)
                    gcol = wk_pool.tile([P, 1], f32, tag="gcol")
                    nc.scalar.activation(gcol[:], gps[:], AF.Sigmoid)
                    # v = ve2 * sigmoid(g) + v_proj (ve2 pre-scaled by 2)
                    nc.vector.scalar_tensor_tensor(
                        out=vsb[:, tt, :], in0=ve2_sb[:, tt, :], scalar=gcol[:],
                        in1=tp[:], op0=ALU.mult, op1=ALU.add,
                    )

            def wave_tails(items):
                t1 = []
                for (head, sl, ps) in items:
                    if head == GH + 1:
                        v_tail(head, sl, ps)
                        continue
                    # rope: qr = ps*cc + swap(ps)*ss  (ss carries the sign)
                    qr = w3_pool.tile([P, 512], f32, tag="qr")
                    nc.vector.tensor_mul(qr[:], ps[:], cc_sb[:, sl])
                    qs = wk_pool.tile([P, 512], f32, tag="qs")
                    nc.vector.tensor_mul(qs[0:64, :], ps[64:128, :],
                                         ss_sb[0:64, sl])
                    nc.vector.tensor_mul(qs[64:128, :], ps[0:64, :],
                                         ss_sb[64:128, sl])
                    nc.vector.tensor_add(qr[:], qr[:], qs[:])
                    q2 = wk_pool.tile([P, 512], bf16, tag="q2")
                    nc.gpsimd.tensor_mul(q2[:], qr[:], qr[:])
                    t1.append((head, sl, qr, q2))
                ssqs = []
                for (head, sl, qr, q2) in t1:
                    ssq = pp.tile([1, 512], f32, tag="pb")
                    nc.tensor.matmul(ssq[:], lhsT=ones_sb[:], rhs=q2[:],
                                     start=True, stop=True)
                    ssqs.append(ssq)
                rows = []
                for (head, sl, qr, q2), ssq in zip(t1, ssqs):
                    srow = w3_pool.tile([1, 512], f32, tag="srow")
                    nc.scalar.activation(srow[:], ssq[:], AF.Sqrt,
                                         bias=eps_sb[0:1, :], scale=1.0 / HD)
                    rows.append(srow)
                rrs = []
                for (head, sl, qr, q2), srow in zip(t1, rows):
                    rr = w3_pool.tile([1, 512], f32, tag="rr")
                    nc.vector.reciprocal_approx_fast(rr[:], srow[:])
                    if head == 0:
                        # fold the 1/sqrt(d) score scale into k-hat
                        nc.vector.tensor_scalar_mul(rr[:], rr[:], ISQ)
                    rrs.append(rr)
                rrbs = []
                for (head, sl, qr, q2), rr in zip(t1, rrs):
                    rrb = pp.tile([P, 512], f32, tag="pb")
                    nc.tensor.matmul(rrb[:], lhsT=ones1f[:], rhs=rr[:],
                                     start=True, stop=True)
                    rrbs.append(rrb)
                for (head, sl, qr, q2), rrb in zip(t1, rrbs):
                    dest = khat[:, sl] if head == 0 else qhat[:, head - 1, sl]
                    nc.vector.tensor_mul(dest, qr[:], rrb[:])

            groups = [(head, ts_) for head in range(GH + 2)
                      for ts_ in range(TS)]
            prev_items = None
            for w0 in range(0, len(groups), 3):
                items = wave_mms(groups[w0:w0 + 3])
                if prev_items:
                    wave_tails(prev_items)
                prev_items = items
            wave_tails(prev_items)

            CO = C_ // 512  # output column chunks
            # All 4 q-heads are fused into one 512-wide moving operand:
            # scores / exp / den / PV are each ONE N=512 instruction per
            # (qi, kt), so LDWEIGHTS fully hides under the matmul stream.
            denps = {}
            yps = {}
            yus = {}
            rds = {}

            def attn_scores_k(qi, kk):
                ktc = min(WT + 1, NT - qi)
                qs4 = qhat[:, :, qi * P:(qi + 1) * P]   # [d, (h, q)] = 512 wide
                kt = qi + kk
                sp = pp.tile([P, GH * P], f32, tag="pb")
                masked = (kk == 0) or (kk == WT and ktc == WT + 1)
                nc.tensor.matmul(
                    sp[:], lhsT=khat[:, kt * P:(kt + 1) * P], rhs=qs4,
                    start=True, stop=not masked,
                )
                if masked:
                    # band-mask bias (-3e4 outside band): psum += bias.T @ I_rep
                    nc.tensor.matmul(
                        sp[:], lhsT=tlo_sb[:] if kk == 0 else thi_sb[:],
                        rhs=idr_sb[:], start=False, stop=True,
                    )
                pt = aw.tile([P, GH * P], bf16, tag="pT")
                nc.scalar.activation(pt[:], sp[:], AF.Exp)
                return pt

            def attn_pv_k(qi, kk, pt):
                ktc = min(WT + 1, NT - qi)
                if kk == 0:
                    denps[qi] = pp.tile([1, GH * P], f32, tag="pb",
                                        name=f"denp{qi}")
                    yps[qi] = pp.tile([P, GH * P], f32, tag="pb",
                                      name=f"yp{qi}")
                kt = qi + kk
                nc.tensor.matmul(
                    denps[qi][:], lhsT=ones_sb[:], rhs=pt[:],
                    start=(kk == 0), stop=(kk == ktc - 1),
                )
                nc.tensor.matmul(
                    yps[qi][:], lhsT=vsb[:, kt, :], rhs=pt[:],
                    start=(kk == 0), stop=(kk == ktc - 1),
                )
                if kk == ktc - 1:
                    yut = yu_pool.tile([P, GH * P], f32, tag="yu")
                    nc.vector.tensor_copy(yut[:], yps[qi][:])
                    yus[qi] = yut
                    rd = wk_pool.tile([1, GH * P], f32, tag="rd")
                    nc.vector.reciprocal_approx_fast(rd[:], denps[qi][:])
                    rds[qi] = rd

            def attn_out(qi):
                qsl = slice(qi * P, (qi + 1) * P)
                rdb = pp.tile([P, GH * P], f32, tag="pb")
                nc.tensor.matmul(rdb[:], lhsT=ones1f[:], rhs=rds[qi][:],
                                 start=True, stop=True)
                yq = op_pool.tile([P, GH * P], bf16, tag="yq")
                nc.vector.tensor_mul(yq[:], yus[qi][:], rdb[:])
                for co in range(CO):
                    osl = slice(co * 512, co * 512 + 512)
                    ops = pp.tile([P, 512], f32, tag="pb")
                    for h in range(GH):
                        nc.tensor.matmul(
                            ops[:], lhsT=yq[:, h * P:(h + 1) * P],
                            rhs=wo_sb[:, h, osl],
                            start=(h == 0), stop=(h == GH - 1),
                        )
                    ob = op_pool.tile([P, 512], f32, tag="ob")
                    nc.vector.tensor_copy(out=ob[:], in_=ops[:])
                    nc.sync.dma_start(out=out_d[qsl, osl], in_=ob[:])

            pv_queue = deque()
            done_out = set()
            out_ready = deque()
            for qi in range(NT):
                ktc = min(WT + 1, NT - qi)
                for kk in range(ktc):
                    pt = attn_scores_k(qi, kk)
                    if len(pv_queue) >= 2:
                        attn_pv_k(*pv_queue.popleft())
                    pv_queue.append((qi, kk, pt))
                    # emit out-proj one iteration after its recip is queued,
                    # so the PE never waits on the denominator chain
                    if out_ready and out_ready[0][1] <= 0:
                        done_out.add(out_ready[0][0])
                        attn_out(out_ready.popleft()[0])
                    out_ready = deque([(q, age - 1) for q, age in out_ready])
                    if qi > 0 and (qi - 1) in rds and (qi - 1) not in done_out \
                            and all(q != qi - 1 for q, _ in out_ready):
                        out_ready.append((qi - 1, 1))
            while pv_queue:
                attn_pv_k(*pv_queue.popleft())
            for qi in range(NT):
                if qi not in done_out:
                    attn_out(qi)

    return nc


def _get_program(T_=T, C_=C, win=WINDOW):
    key = (T_, C_, win)
    if key not in _PROGRAM_CACHE:
        nc = build_program(T_, C_, win)
        nc.finalize()
        _PROGRAM_CACHE[key] = nc
    return _PROGRAM_CACHE[key]


def make_in_maps(x, ve, cos, sin, Wq, Wk, Wv, Wg, Wo):
    """Build the 8 per-core input dicts (host-side sharding/layout prep)."""
    cosT = np.ascontiguousarray(cos[:, 0, :].T).astype(np.float32)  # [64, T]
    sinT = np.ascontiguousarray(sin[:, 0, :].T).astype(np.float32)
    cc = np.concatenate([cosT, cosT], axis=0)            # [128, T]
    ss = np.concatenate([sinT, -sinT], axis=0)           # [128, T]
    # additive mask biases for the S^T diagonal/far tiles, pre-transposed
    # (they enter the scores as lhsT with an identity rhs: psum += bias.T)
    neg = np.float32(-30000.0)
    bias_lo = np.where(np.arange(P)[:, None] >= np.arange(P)[None, :], 0.0, neg)
    bias_hi = np.where(np.arange(P)[:, None] < np.arange(P)[None, :], 0.0, neg)
    tlo = np.ascontiguousarray(bias_lo.T).astype(BF16)
    thi = np.ascontiguousarray(bias_hi.T).astype(BF16)
    identr = np.tile(np.eye(P, dtype=np.float32), (1, GH)).astype(BF16)
    identf = np.eye(P, dtype=np.float32)

    in_maps = []
    for core in range(N_CORES):
        b, g = divmod(core, N_KV)
        in_maps.append({
            "xT": np.ascontiguousarray(x[b].T).astype(BF16),
            "wq": Wq[:, g * GH * HD:(g + 1) * GH * HD].astype(BF16),
            "wk": Wk[:, g * HD:(g + 1) * HD].astype(BF16),
            "wv": Wv[:, g * HD:(g + 1) * HD].astype(BF16),
            "wg": np.ascontiguousarray(Wg[:, g:g + 1]).astype(BF16),
            "ve2": (2.0 * ve[b][:, g * HD:(g + 1) * HD]).astype(BF16),
            "wo": Wo[g * GH * HD:(g + 1) * GH * HD, :].astype(BF16),
            "cc": cc.astype(BF16), "ss": ss.astype(BF16),
            "tlo": tlo, "thi": thi, "identr": identr, "identf": identf,
        })
    return in_maps


def kernel(x, ve, cos, sin, Wq, Wk, Wv, Wg, Wo, window):
    assert int(window) == WINDOW and x.shape == (B, T, C)
    from concourse.bass_utils import run_bass_kernel_spmd

    nc = _get_program()
    in_maps = make_in_maps(x, ve, cos, sin, Wq, Wk, Wv, Wg, Wo)
    res = run_bass_kernel_spmd(nc, in_maps, core_ids=list(range(N_CORES)))
    out = np.zeros((B, T, C), dtype=np.float32)
    for core in range(N_CORES):
        b = core // N_KV
        out[b] += res.results[core]["out"]
    return out



# revision 6
# speedup vs baseline: 1.0130x; 1.0130x over previous
"""Trainium2 Bass kernel for a GQA sliding-window attention layer.

Reference computation (B=2, T=2048, C=2048, 16 Q heads / 4 KV heads, d=128):
    q = x @ Wq; k = x @ Wk; v = x @ Wv (+ sigmoid-gated value embedding)
    q, k = rmsnorm(rope(q)), rmsnorm(rope(k))
    scores masked to the band 0 <= j - i < window (=1024), softmax over j
    out = (p @ v) @ Wo

Sharding: 8 cores = 2 batches x 4 KV groups.  Each core computes its 4 Q
heads / 1 KV head for one batch and a partial output (its 512-row slice of
the Wo contraction); the host sums the 4 partials per batch (bf16 partials).

v2 changes vs the 377us baseline (all aimed at the PE bottleneck):
  - softmax denominators no longer use 108 ones-matmuls: exp tiles are
    accumulated on DVE/Pool (bf16 chains), then ONE ones[128,128] matmul
    per q-tile both sums over kj and broadcasts 1/den across partitions.
  - band masks are multiplicative 0/1 masks applied on Pool after exp
    (removes 24 PE bias-matmuls).
  - rms row broadcasts are free: the sum-of-squares matmul uses a
    ones[128,128] stationary, so every partition already has the row sum
    (removes 36 broadcast matmuls).
  - projections emitted in waves that reuse one stationary for 2-4
    matmuls; out-proj is h-outer/co-inner; a post-schedule IR pass
    deletes redundant InstLdweights (same weights AP, no sync info).
  - output written in bf16 (halves the 16MB/core output DMA).
"""

import numpy as np
import ml_dtypes
from collections import deque

BF16 = ml_dtypes.bfloat16

B, T, C = 2, 2048, 2048
N_HEAD, N_KV, HD, GATE_CH = 16, 4, 128, 32
WINDOW = 1024
P = 128
GH = N_HEAD // N_KV  # q heads per kv head (= per core)
N_CORES = 8

_PROGRAM_CACHE = {}


def dedup_ldweights(nc):
    """Remove InstLdweights that reload the stationary already in the PE.

    Safe subset only: the candidate must have identical (memref, offset,
    pattern, dtype) to the previous PE weight load in the same block, and
    carry no semaphore waits/updates (sync_info None).  Content safety:
    between two identical loads with no other InstLdweights in between, no
    producer rewrites the weights region (weight tiles are write-once per
    buffer rotation, and any rotation reuse has many other loads between).
    """

    def sig(inst):
        ap = inst.ins[0]
        try:
            return (ap.memref, ap.offset, str(ap.ap), str(ap.dtype))
        except Exception:
            return None

    removed = 0
    for f in nc.m.functions:
        for b in f.blocks:
            il = b.instructions
            last = None
            doomed = []
            for inst in il:
                tn = type(inst).__name__
                if tn == "InstLdweights":
                    s = sig(inst)
                    if s is not None and s == last and inst.sync_info is None:
                        doomed.append(inst)
                    else:
                        last = s
            for inst in doomed:
                il.remove(inst)
                removed += 1
    return removed


def build_program(T_=T, C_=C, win=WINDOW, dedup=True):
    import concourse.mybir as mybir
    import concourse.tile as tile
    from concourse import bacc

    dt = mybir.dt
    f32 = dt.float32
    bf16 = dt.bfloat16
    AF = mybir.ActivationFunctionType
    ALU = mybir.AluOpType

    NT = T_ // P          # token tiles
    KT = C_ // P          # contraction tiles
    WT = win // P         # window tiles
    TS = T_ // 512        # 512-wide token slices

    nc = bacc.Bacc()

    xT = nc.declare_dram_parameter("xT", [C_, T_], bf16, isOutput=False)
    wq = nc.declare_dram_parameter("wq", [C_, GH * HD], bf16, isOutput=False)
    wk = nc.declare_dram_parameter("wk", [C_, HD], bf16, isOutput=False)
    wv = nc.declare_dram_parameter("wv", [C_, HD], bf16, isOutput=False)
    wg = nc.declare_dram_parameter("wg", [GATE_CH, 1], bf16, isOutput=False)
    ve2 = nc.declare_dram_parameter("ve2", [T_, HD], bf16, isOutput=False)
    wo = nc.declare_dram_parameter("wo", [GH * HD, C_], bf16, isOutput=False)
    ccd = nc.declare_dram_parameter("cc", [P, T_], bf16, isOutput=False)
    ssd = nc.declare_dram_parameter("ss", [P, T_], bf16, isOutput=False)
    mlo = nc.declare_dram_parameter("mlo", [P, GH * P], bf16, isOutput=False)
    mhi = nc.declare_dram_parameter("mhi", [P, GH * P], bf16, isOutput=False)
    idb = nc.declare_dram_parameter("identb", [P, P], bf16, isOutput=False)
    out_d = nc.declare_dram_parameter("out", [T_, C_], bf16, isOutput=True)

    with tile.TileContext(nc) as tc:
        with (
            tc.tile_pool(name="singles", bufs=1) as sg,
            tc.tile_pool(name="rope", bufs=3) as rp,
            tc.tile_pool(name="rms", bufs=2) as rm,
            tc.tile_pool(name="attw", bufs=7) as aw,
            tc.tile_pool(name="accs", bufs=2) as acp,
            tc.tile_pool(name="yqp", bufs=2) as yqp,
            tc.tile_pool(name="outp", bufs=4) as op_pool,
            tc.tile_pool(name="psA", bufs=2, space="PSUM") as ppA,   # scores/aux (2 banks)
            tc.tile_pool(name="psY", bufs=2, space="PSUM") as ppY,   # PV accum (2 banks)
            tc.tile_pool(name="psB", bufs=4, space="PSUM") as ppB,   # den/out + wave (4 banks)
        ):
            # ---- input DMAs, priority order ----------------------------
            # tiny constants + cc/ss first (k-tails read them ~20us in),
            # then xT/weight tiles kt-by-kt so wave1 can chase the stream.
            wg_sb = sg.tile([GATE_CH, 1], bf16, tag="wg")
            nc.sync.dma_start(out=wg_sb[:], in_=wg[:])
            mlo_sb = sg.tile([P, GH * P], bf16, tag="mlo")
            nc.sync.dma_start(out=mlo_sb[:], in_=mlo[:])
            mhi_sb = sg.tile([P, GH * P], bf16, tag="mhi")
            nc.sync.dma_start(out=mhi_sb[:], in_=mhi[:])
            idb_sb = sg.tile([P, P], bf16, tag="idb")
            nc.sync.dma_start(out=idb_sb[:], in_=idb[:])
            cc_sb = sg.tile([P, T_], bf16, tag="cc")
            nc.sync.dma_start(out=cc_sb[:], in_=ccd[:])
            ss_sb = sg.tile([P, T_], bf16, tag="ss")
            nc.sync.dma_start(out=ss_sb[:], in_=ssd[:])

            xt = []
            wq_sb = sg.tile([P, KT, GH * HD], bf16, tag="wq")
            wk_sb = sg.tile([P, KT, HD], bf16, tag="wk")
            wv_sb = sg.tile([P, KT, HD], bf16, tag="wv")
            wqr = wq.rearrange("(o p) n -> p o n", p=P)
            wkr = wk.rearrange("(o p) n -> p o n", p=P)
            wvr = wv.rearrange("(o p) n -> p o n", p=P)
            for kt in range(KT):
                t_ = sg.tile([P, T_], bf16, tag=f"xt{kt}")
                nc.sync.dma_start(out=wk_sb[:, kt, :], in_=wkr[:, kt, :])
                nc.sync.dma_start(out=t_[:], in_=xT[kt * P:(kt + 1) * P, :])
                xt.append(t_)
                nc.sync.dma_start(out=wv_sb[:, kt, :], in_=wvr[:, kt, :])
                nc.sync.dma_start(out=wq_sb[:, kt, :], in_=wqr[:, kt, :])
            ve2_sb = sg.tile([P, NT, HD], bf16, tag="ve2")
            nc.sync.dma_start(out=ve2_sb[:], in_=ve2.rearrange("(o p) d -> p o d", p=P))
            wo_sb = sg.tile([P, GH, C_], bf16, tag="wo")
            nc.sync.dma_start(out=wo_sb[:], in_=wo.rearrange("(o p) n -> p o n", p=P))

            ones_sb = sg.tile([P, P], bf16, tag="ones128")
            nc.vector.memset(ones_sb[:], 1.0)
            epsq_sb = sg.tile([P, 1], f32, tag="epsq")
            nc.vector.memset(epsq_sb[:], 1e-6)
            epsk_sb = sg.tile([P, 1], f32, tag="epsk")
            nc.vector.memset(epsk_sb[:], HD * 1e-6)

            # persistent intermediates
            qhat = sg.tile([P, GH, T_], bf16, tag="qhat")   # normalized roped q, [d, h, t]
            khat = sg.tile([P, T_], bf16, tag="khat")       # normalized roped k * isq
            vsb = sg.tile([P, NT, HD], bf16, tag="vsb")     # gated v, [tok, tt, d]

            # ---- projections: waves of 6 psum groups -------------------
            # head ids: 0=k, 1=v, 2..5=q0..q3.  Wave order is chosen so
            # that after wave3's tails, attention qi=0..7 is fully
            # unlocked (khat/vsb complete, all q heads for tokens <1024) —
            # wave4's tails then overlap the start of attention.
            WAVES = [
                [(0, 0), (0, 1), (0, 2), (0, 3), (1, 0), (1, 1)],
                [(1, 2), (1, 3), (2, 0), (2, 1), (2, 2), (2, 3)],
                [(3, 0), (3, 1), (4, 0), (4, 1), (5, 0), (5, 1)],
                [(3, 2), (3, 3), (4, 2), (4, 3), (5, 2), (5, 3)],
            ]

            def w_ap(head, kt):
                if head == 0:
                    return wk_sb[:, kt, :]
                if head == 1:
                    return wv_sb[:, kt, :]
                return wq_sb[:, kt, (head - 2) * HD:(head - 1) * HD]

            def wave_mms(wave):
                items = []
                for gi, (head, ts_) in enumerate(wave):
                    sl = slice(ts_ * 512, ts_ * 512 + 512)
                    pool_, tag_ = (ppY, "yp") if gi < 2 else (ppB, "pb")
                    ps = pool_.tile([P, 512], f32, tag=tag_, name=f"ps{head}_{ts_}")
                    items.append((head, sl, ps))
                for kt in range(KT):
                    # group by head so consecutive matmuls share one stationary
                    for gi, (head, ts_) in enumerate(wave):
                        nc.tensor.matmul(
                            items[gi][2][:], lhsT=w_ap(head, kt),
                            rhs=xt[kt][:, items[gi][1]],
                            start=(kt == 0), stop=(kt == KT - 1),
                        )
                return items

            def v_tail(sl, ps):
                # vT psum [d, tok] -> bf16 sbuf, PE-transpose each 128-tok
                # block to [tok, d], add sigmoid-gated ve.
                vt = rp.tile([P, 512], bf16, tag="vt")
                nc.scalar.copy(out=vt[:], in_=ps[:])
                for i in range(4):
                    tt = sl.start // P + i
                    tsl = slice(tt * P, (tt + 1) * P)
                    tp = ppA.tile([P, P], bf16, tag="sc", name=f"tp{tt}")
                    nc.tensor.transpose(tp[:], vt[:, i * P:(i + 1) * P], idb_sb[:])
                    gps = ppA.tile([P, 1], f32, tag="sc", name=f"gp{tt}")
                    nc.tensor.matmul(gps[:], lhsT=xt[0][0:GATE_CH, tsl],
                                     rhs=wg_sb[:], start=True, stop=True)
                    gcol = rp.tile([P, 1], f32, tag="gcol")
                    nc.scalar.activation(gcol[:], gps[:], AF.Sigmoid)
                    # v = ve2 * sigmoid(g) + v_proj (ve2 pre-scaled by 2)
                    nc.vector.scalar_tensor_tensor(
                        out=vsb[:, tt, :], in0=ve2_sb[:, tt, :], scalar=gcol[:],
                        in1=tp[:], op0=ALU.mult, op1=ALU.add,
                    )

            def qk_tail(head, sl, ps):
                # rope: qr = ps*cc + swap(ps)*ss  (ss carries the sign)
                qr = rp.tile([P, 512], f32, tag="qr")
                nc.vector.tensor_mul(qr[:], ps[:], cc_sb[:, sl])
                qs = rp.tile([P, 512], f32, tag="qs")
                nc.vector.tensor_mul(qs[0:64, :], ps[64:128, :], ss_sb[0:64, sl])
                nc.vector.tensor_mul(qs[64:128, :], ps[0:64, :], ss_sb[64:128, sl])
                nc.vector.tensor_add(qr[:], qr[:], qs[:])
                q2 = rp.tile([P, 512], bf16, tag="q2")
                nc.gpsimd.tensor_mul(q2[:], qr[:], qr[:])
                # ssq with ones[128,128] stationary: every partition gets the
                # column sum -> rms row already broadcast, no extra matmul.
                ssq = ppA.tile([P, 512], f32, tag="sc", name=f"ssq{head}_{sl.start}")
                nc.tensor.matmul(ssq[:], lhsT=ones_sb[:], rhs=q2[:],
                                 start=True, stop=True)
                srow = rm.tile([P, 512], f32, tag="srow")
                if head == 0:
                    # k: fold the 1/sqrt(d) score scale into k-hat:
                    # 1/sqrt(ssq + HD*eps) = isq/sqrt(ms + eps)
                    nc.scalar.activation(srow[:], ssq[:], AF.Sqrt,
                                         bias=epsk_sb[:], scale=1.0)
                else:
                    nc.scalar.activation(srow[:], ssq[:], AF.Sqrt,
                                         bias=epsq_sb[:], scale=1.0 / HD)
                rr = rm.tile([P, 512], f32, tag="rr")
                nc.vector.reciprocal_approx_fast(rr[:], srow[:])
                dest = khat[:, sl] if head == 0 else qhat[:, head - 2, sl]
                nc.vector.tensor_mul(dest, qr[:], rr[:])

            def wave_tails(items):
                for (head, sl, ps) in items:
                    if head == 1:
                        v_tail(sl, ps)
                    else:
                        qk_tail(head, sl, ps)

            prev_items = None
            for wave in WAVES:
                items = wave_mms(wave)
                if prev_items:
                    wave_tails(prev_items)
                prev_items = items
            wave_tails(prev_items)

            # ---- attention + out-proj ---------------------------------
            # S^T tiles [kj, (h,q)] fused across the 4 heads (512 wide).
            CO = C_ // 512
            ISQ_NONE = None  # isq folded into khat

            yps = {}
            accA = {}
            accB = {}
            accb = {}
            denp = {}
            rds = {}
            yqs = {}

            def ktc_of(qi):
                return min(WT + 1, NT - qi)

            def emit_scores(qi, kk):
                kt = qi + kk
                qs4 = qhat[:, :, qi * P:(qi + 1) * P]
                sp = ppA.tile([P, GH * P], f32, tag="sc", name=f"sp{qi}_{kk}")
                nc.tensor.matmul(sp[:], lhsT=khat[:, kt * P:(kt + 1) * P],
                                 rhs=qs4, start=True, stop=True)
                pt = aw.tile([P, GH * P], bf16, tag="pt")
                nc.scalar.activation(pt[:], sp[:], AF.Exp)
                masked_lo = (kk == 0)
                masked_hi = (kk == WT and ktc_of(qi) == WT + 1)
                if masked_lo or masked_hi:
                    ptm = aw.tile([P, GH * P], bf16, tag="pt")
                    nc.gpsimd.tensor_mul(
                        ptm[:], pt[:], mlo_sb[:] if masked_lo else mhi_sb[:])
                    pt = ptm
                return pt

            def emit_pv(qi, kk, pt):
                ktc = ktc_of(qi)
                kt = qi + kk
                if kk == 0:
                    yps[qi] = ppY.tile([P, GH * P], f32, tag="yp", name=f"yp{qi}")
                    accA[qi] = pt   # bf16 partial-sum chain heads
                elif kk == 1:
                    accB[qi] = pt
                elif kk % 2 == 0:
                    if kk == 2:
                        t_ = acp.tile([P, GH * P], bf16, tag="accA")
                        nc.vector.tensor_add(t_[:], accA[qi][:], pt[:])
                        accA[qi] = t_
                    else:
                        nc.vector.tensor_add(accA[qi][:], accA[qi][:], pt[:])
                else:
                    if kk == 3:
                        t_ = acp.tile([P, GH * P], bf16, tag="accB")
                        nc.gpsimd.tensor_add(t_[:], accB[qi][:], pt[:])
                        accB[qi] = t_
                    else:
                        nc.gpsimd.tensor_add(accB[qi][:], accB[qi][:], pt[:])
                nc.tensor.matmul(
                    yps[qi][:], lhsT=vsb[:, kt, :], rhs=pt[:],
                    start=(kk == 0), stop=(kk == ktc - 1),
                )

            def emit_merge(qi):
                if qi not in accB:
                    accb[qi] = accA[qi]
                else:
                    t_ = acp.tile([P, GH * P], bf16, tag="accM")
                    nc.vector.tensor_add(t_[:], accA[qi][:], accB[qi][:])
                    accb[qi] = t_

            def emit_den(qi):
                # one matmul: sums over kj AND broadcasts across partitions
                denp[qi] = ppB.tile([P, GH * P], f32, tag="pb", name=f"dn{qi}")
                nc.tensor.matmul(denp[qi][:], lhsT=ones_sb[:], rhs=accb[qi][:],
                                 start=True, stop=True)
                rd = rm.tile([P, GH * P], f32, tag="rd")
                nc.vector.reciprocal_approx_fast(rd[:], denp[qi][:])
                rds[qi] = rd

            def emit_yq(qi):
                yq = yqp.tile([P, GH * P], bf16, tag="yq")
                nc.vector.tensor_mul(yq[:], yps[qi][:], rds[qi][:])
                yqs[qi] = yq

            def emit_out(qi):
                qsl = slice(qi * P, (qi + 1) * P)
                yq = yqs[qi]
                ops = [ppB.tile([P, 512], f32, tag="pb", name=f"op{qi}_{co}")
                       for co in range(CO)]
                for h in range(GH):
                    for co in range(CO):
                        nc.tensor.matmul(
                            ops[co][:], lhsT=yq[:, h * P:(h + 1) * P],
                            rhs=wo_sb[:, h, co * 512:(co + 1) * 512],
                            start=(h == 0), stop=(h == GH - 1),
                        )
                for co in range(CO):
                    ob = op_pool.tile([P, 512], bf16, tag="ob")
                    nc.scalar.copy(out=ob[:], in_=ops[co][:])
                    nc.sync.dma_start(out=out_d[qsl, co * 512:(co + 1) * 512],
                                      in_=ob[:])

            # main interleaved loop: scores run 2 ahead of PV; the deferred
            # den/yq/out stages of qi run early in qi+1's stream so the PE
            # never waits on the DVE/ACT chains.
            pv_queue = deque()
            deferred = deque()
            for qi in range(NT):
                ktc = ktc_of(qi)
                for kk in range(ktc):
                    pt = emit_scores(qi, kk)
                    if len(pv_queue) >= 2:
                        q_, k_, p_ = pv_queue.popleft()
                        emit_pv(q_, k_, p_)
                        if k_ == ktc_of(q_) - 1:
                            emit_merge(q_)
                            deferred.append((emit_den, q_))
                            deferred.append((emit_yq, q_))
                            deferred.append((emit_out, q_))
                    pv_queue.append((qi, kk, pt))
                    if deferred and kk >= 2:
                        fn, q_ = deferred.popleft()
                        fn(q_)
            while pv_queue:
                q_, k_, p_ = pv_queue.popleft()
                emit_pv(q_, k_, p_)
                if k_ == ktc_of(q_) - 1:
                    emit_merge(q_)
                    deferred.append((emit_den, q_))
                    deferred.append((emit_yq, q_))
                    deferred.append((emit_out, q_))
            while deferred:
                fn, q_ = deferred.popleft()
                fn(q_)

    if dedup:
        n = dedup_ldweights(nc)
        import logging
        logging.getLogger(__name__).info(f"dedup_ldweights removed {n}")
    return nc


def _get_program(T_=T, C_=C, win=WINDOW, dedup=True):
    key = (T_, C_, win, dedup)
    if key not in _PROGRAM_CACHE:
        nc = build_program(T_, C_, win, dedup=dedup)
        nc.finalize()
        _PROGRAM_CACHE[key] = nc
    return _PROGRAM_CACHE[key]


def make_in_maps(x, ve, cos, sin, Wq, Wk, Wv, Wg, Wo):
    """Build the 8 per-core input dicts (host-side sharding/layout prep)."""
    cosT = np.ascontiguousarray(cos[:, 0, :].T).astype(np.float32)  # [64, T]
    sinT = np.ascontiguousarray(sin[:, 0, :].T).astype(np.float32)
    cc = np.concatenate([cosT, cosT], axis=0)            # [128, T]
    ss = np.concatenate([sinT, -sinT], axis=0)           # [128, T]
    # multiplicative 0/1 band masks in S^T coords [kj, q], tiled per head.
    # diag tile (kk==0): keep kj >= q ; far tile (kk==WT): keep kj < q
    kj = np.arange(P)[:, None]
    q = np.arange(P)[None, :]
    m_lo = (kj >= q).astype(np.float32)
    m_hi = (kj < q).astype(np.float32)
    mlo = np.tile(m_lo, (1, GH)).astype(BF16)
    mhi = np.tile(m_hi, (1, GH)).astype(BF16)
    identb = np.eye(P, dtype=np.float32).astype(BF16)

    in_maps = []
    for core in range(N_CORES):
        b, g = divmod(core, N_KV)
        in_maps.append({
            "xT": np.ascontiguousarray(x[b].T).astype(BF16),
            "wq": Wq[:, g * GH * HD:(g + 1) * GH * HD].astype(BF16),
            "wk": Wk[:, g * HD:(g + 1) * HD].astype(BF16),
            "wv": Wv[:, g * HD:(g + 1) * HD].astype(BF16),
            "wg": np.ascontiguousarray(Wg[:, g:g + 1]).astype(BF16),
            "ve2": (2.0 * ve[b][:, g * HD:(g + 1) * HD]).astype(BF16),
            "wo": Wo[g * GH * HD:(g + 1) * GH * HD, :].astype(BF16),
            "cc": cc.astype(BF16), "ss": ss.astype(BF16),
            "mlo": mlo, "mhi": mhi, "identb": identb,
        })
    return in_maps


def kernel(x, ve, cos, sin, Wq, Wk, Wv, Wg, Wo, window):
    assert int(window) == WINDOW and x.shape == (B, T, C)
    from concourse.bass_utils import run_bass_kernel_spmd

    nc = _get_program()
    in_maps = make_in_maps(x, ve, cos, sin, Wq, Wk, Wv, Wg, Wo)
    res = run_bass_kernel_spmd(nc, in_maps, core_ids=list(range(N_CORES)))
    out = np.zeros((B, T, C), dtype=np.float32)
    for core in range(N_CORES):
        b = core // N_KV
        out[b] += res.results[core]["out"].astype(np.float32)
    return out


# revision 18
# speedup vs baseline: 1.2749x; 1.2585x over previous
"""Trainium2 Bass kernel for a GQA sliding-window attention layer.

Reference computation (B=2, T=2048, C=2048, 16 Q heads / 4 KV heads, d=128):
    q = x @ Wq; k = x @ Wk; v = x @ Wv (+ sigmoid-gated value embedding)
    q, k = rmsnorm(rope(q)), rmsnorm(rope(k))
    scores masked to the band 0 <= j - i < window (=1024), softmax over j
    out = (p @ v) @ Wo

Sharding: 8 cores = 2 batches x 4 KV groups.  Each core computes its 4 Q
heads / 1 KV head for one batch and a partial output (its 512-row slice of
the Wo contraction); the host sums the 4 partials per batch (bf16 partials).

v2 changes vs the 377us baseline (all aimed at the PE bottleneck):
  - softmax denominators no longer use 108 ones-matmuls: exp tiles are
    accumulated on DVE/Pool (bf16 chains), then ONE ones[128,128] matmul
    per q-tile both sums over kj and broadcasts 1/den across partitions.
  - band masks are multiplicative 0/1 masks applied on Pool after exp
    (removes 24 PE bias-matmuls).
  - rms row broadcasts are free: the sum-of-squares matmul uses a
    ones[128,128] stationary, so every partition already has the row sum
    (removes 36 broadcast matmuls).
  - projections emitted in waves that reuse one stationary for 2-4
    matmuls; out-proj is h-outer/co-inner; a post-schedule IR pass
    deletes redundant InstLdweights (same weights AP, no sync info).
  - output written in bf16 (halves the 16MB/core output DMA).
"""

import numpy as np
import ml_dtypes
from collections import deque

BF16 = ml_dtypes.bfloat16

B, T, C = 2, 2048, 2048
N_HEAD, N_KV, HD, GATE_CH = 16, 4, 128, 32
WINDOW = 1024
P = 128
GH = N_HEAD // N_KV  # q heads per kv head (= per core)
N_CORES = 8

_PROGRAM_CACHE = {}


def dedup_ldweights(nc):
    """Remove InstLdweights that reload the stationary already in the PE.

    Safe subset only: the candidate must have identical (memref, offset,
    pattern, dtype) to the previous PE weight load in the same block, and
    carry no semaphore waits/updates (sync_info None).  Content safety:
    between two identical loads with no other InstLdweights in between, no
    producer rewrites the weights region (weight tiles are write-once per
    buffer rotation, and any rotation reuse has many other loads between).
    """

    def sig(inst):
        ap = inst.ins[0]
        try:
            return (ap.memref, ap.offset, str(ap.ap), str(ap.dtype))
        except Exception:
            return None

    removed = 0
    for f in nc.m.functions:
        for b in f.blocks:
            il = b.instructions
            last = None
            doomed = []
            for inst in il:
                tn = type(inst).__name__
                if tn == "InstLdweights":
                    s = sig(inst)
                    if s is not None and s == last and inst.sync_info is None:
                        doomed.append(inst)
                    else:
                        last = s
            for inst in doomed:
                il.remove(inst)
                removed += 1
    return removed


def build_program(T_=T, C_=C, win=WINDOW, dedup=True):
    import concourse.mybir as mybir
    import concourse.tile as tile
    from concourse import bacc

    dt = mybir.dt
    f32 = dt.float32
    bf16 = dt.bfloat16
    AF = mybir.ActivationFunctionType
    ALU = mybir.AluOpType

    NT = T_ // P          # token tiles
    KT = C_ // P          # contraction tiles
    WT = win // P         # window tiles
    TS = T_ // 512        # 512-wide token slices

    nc = bacc.Bacc()

    xT = nc.declare_dram_parameter("xT", [C_, T_], bf16, isOutput=False)
    wq = nc.declare_dram_parameter("wq", [C_, GH * HD], bf16, isOutput=False)
    wk = nc.declare_dram_parameter("wk", [C_, HD], bf16, isOutput=False)
    wv = nc.declare_dram_parameter("wv", [C_, HD], bf16, isOutput=False)
    wg = nc.declare_dram_parameter("wg", [GATE_CH, 1], bf16, isOutput=False)
    ve2 = nc.declare_dram_parameter("ve2", [T_, HD], bf16, isOutput=False)
    wo = nc.declare_dram_parameter("wo", [GH * HD, C_], bf16, isOutput=False)
    ccd = nc.declare_dram_parameter("cc", [P, T_], bf16, isOutput=False)
    ssd = nc.declare_dram_parameter("ss", [P, T_], bf16, isOutput=False)
    mlo = nc.declare_dram_parameter("mlo", [P, GH * P], bf16, isOutput=False)
    mhi = nc.declare_dram_parameter("mhi", [P, GH * P], bf16, isOutput=False)
    idb = nc.declare_dram_parameter("identb", [P, P], bf16, isOutput=False)
    out_d = nc.declare_dram_parameter("out", [T_, C_], bf16, isOutput=True)

    with tile.TileContext(nc) as tc:
        with (
            tc.tile_pool(name="singles", bufs=1) as sg,
            tc.tile_pool(name="rope", bufs=3) as rp,
            tc.tile_pool(name="rms", bufs=2) as rm,
            tc.tile_pool(name="attw", bufs=7) as aw,
            tc.tile_pool(name="accs", bufs=2) as acp,
            tc.tile_pool(name="yqp", bufs=2) as yqp,
            tc.tile_pool(name="outp", bufs=4) as op_pool,
            tc.tile_pool(name="psA", bufs=3, space="PSUM") as ppA,   # scores/aux (3 banks)
            tc.tile_pool(name="psY", bufs=2, space="PSUM") as ppY,   # PV accum (2 banks)
            tc.tile_pool(name="psO", bufs=2, space="PSUM") as ppO,   # out-proj pairs (2 banks)
            tc.tile_pool(name="psD", bufs=1, space="PSUM") as ppD,   # den (1 bank)
        ):
            # ---- input DMAs, priority order ----------------------------
            # few, large DMAs (per-descriptor cost is ~0.4-0.6us on the
            # queue): tiny consts, whole wk/wv, then the 16 xT tiles that
            # wave1 chases, then cc/ss (first k-tail needs them ~30us in),
            # wq in 4 chunks (wave2 chases), ve2, wo (needed last).
            wg_sb = sg.tile([GATE_CH, 1], bf16, tag="wg")
            nc.sync.dma_start(out=wg_sb[:], in_=wg[:])
            mlo_sb = sg.tile([P, GH * P], bf16, tag="mlo")
            nc.sync.dma_start(out=mlo_sb[:], in_=mlo[:])
            mhi_sb = sg.tile([P, GH * P], bf16, tag="mhi")
            nc.sync.dma_start(out=mhi_sb[:], in_=mhi[:])
            idb_sb = sg.tile([P, P], bf16, tag="idb")
            nc.sync.dma_start(out=idb_sb[:], in_=idb[:])

            wq_sb = sg.tile([P, KT, GH * HD], bf16, tag="wq")
            wk_sb = sg.tile([P, KT, HD], bf16, tag="wk")
            wv_sb = sg.tile([P, KT, HD], bf16, tag="wv")
            wqr = wq.rearrange("(o p) n -> p o n", p=P)
            nc.sync.dma_start(out=wk_sb[:], in_=wk.rearrange("(o p) n -> p o n", p=P))
            nc.sync.dma_start(out=wv_sb[:], in_=wv.rearrange("(o p) n -> p o n", p=P))
            xt = []
            for kt in range(KT):
                t_ = sg.tile([P, T_], bf16, tag=f"xt{kt}")
                nc.sync.dma_start(out=t_[:], in_=xT[kt * P:(kt + 1) * P, :])
                xt.append(t_)
            cc_sb = sg.tile([P, T_], bf16, tag="cc")
            nc.sync.dma_start(out=cc_sb[:], in_=ccd[:])
            ss_sb = sg.tile([P, T_], bf16, tag="ss")
            nc.sync.dma_start(out=ss_sb[:], in_=ssd[:])
            for qc in range(4):
                nc.sync.dma_start(out=wq_sb[:, 4 * qc:4 * (qc + 1), :],
                                  in_=wqr[:, 4 * qc:4 * (qc + 1), :])
            ve2_sb = sg.tile([P, NT, HD], bf16, tag="ve2")
            nc.sync.dma_start(out=ve2_sb[:], in_=ve2.rearrange("(o p) d -> p o d", p=P))
            wo_sb = sg.tile([P, GH, C_], bf16, tag="wo")
            nc.sync.dma_start(out=wo_sb[:], in_=wo.rearrange("(o p) n -> p o n", p=P))

            ones_sb = sg.tile([P, P], bf16, tag="ones128")
            nc.vector.memset(ones_sb[:], 1.0)
            epsq_sb = sg.tile([P, 1], f32, tag="epsq")
            nc.vector.memset(epsq_sb[:], 1e-6)
            epsk_sb = sg.tile([P, 1], f32, tag="epsk")
            nc.vector.memset(epsk_sb[:], HD * 1e-6)

            # persistent intermediates
            qhat = sg.tile([P, GH, T_], bf16, tag="qhat")   # normalized roped q, [d, h, t]
            khat = sg.tile([P, T_], bf16, tag="khat")       # normalized roped k * isq
            vsb = sg.tile([P, NT, HD], bf16, tag="vsb")     # gated v, [tok, tt, d]

            # ---- projections: waves of 5 psum groups -------------------
            # head ids: 0=k, 1=v, 2..5=q0..q3.  Wave order: after wave4's
            # tails, attention qi=0..7 is fully unlocked (khat/vsb done,
            # all q heads for tokens <1024) — wave5's tails overlap the
            # start of attention.  Within a kt, groups sharing a head run
            # back-to-back so dedup removes their LDWEIGHTS.
            WAVES = [
                [(0, 0), (0, 1), (0, 2), (0, 3), (1, 0)],
                [(1, 1), (1, 2), (1, 3), (2, 0), (2, 1)],
                [(3, 0), (3, 1), (4, 0), (4, 1), (5, 0)],
                [(5, 1), (2, 2), (2, 3), (3, 2), (3, 3)],
                [(4, 2), (5, 2), (4, 3), (5, 3)],
            ]
            # wave psums live in yp(2)+op(2)+dn(1) = 5 banks; the previous
            # wave's tails use the sc ring (3 banks) for ssq/transpose/gate.
            WAVE_SLOTS = [(ppY, "yp"), (ppY, "yp"), (ppO, "op"), (ppO, "op"),
                          (ppD, "dn")]

            def w_ap(head, kt):
                if head == 0:
                    return wk_sb[:, kt, :]
                if head == 1:
                    return wv_sb[:, kt, :]
                return wq_sb[:, kt, (head - 2) * HD:(head - 1) * HD]

            def wave_mms(wave):
                items = []
                for gi, (head, ts_) in enumerate(wave):
                    sl = slice(ts_ * 512, ts_ * 512 + 512)
                    pool_, tag_ = WAVE_SLOTS[gi]
                    ps = pool_.tile([P, 512], f32, tag=tag_, name=f"ps{head}_{ts_}")
                    items.append((head, sl, ps))
                for kt in range(KT):
                    # group by head so consecutive matmuls share one stationary
                    for gi, (head, ts_) in enumerate(wave):
                        nc.tensor.matmul(
                            items[gi][2][:], lhsT=w_ap(head, kt),
                            rhs=xt[kt][:, items[gi][1]],
                            start=(kt == 0), stop=(kt == KT - 1),
                        )
                return items

            def v_tail(sl, ps):
                # vT psum [d, tok] -> bf16 sbuf, PE-transpose each 128-tok
                # block to [tok, d], add sigmoid-gated ve.
                vt = rp.tile([P, 512], bf16, tag="vt")
                nc.scalar.copy(out=vt[:], in_=ps[:])
                for i in range(4):
                    tt = sl.start // P + i
                    tsl = slice(tt * P, (tt + 1) * P)
                    tp = ppA.tile([P, P], bf16, tag="sc", name=f"tp{tt}")
                    nc.tensor.transpose(tp[:], vt[:, i * P:(i + 1) * P], idb_sb[:])
                    gps = ppA.tile([P, 1], f32, tag="sc", name=f"gp{tt}")
                    nc.tensor.matmul(gps[:], lhsT=xt[0][0:GATE_CH, tsl],
                                     rhs=wg_sb[:], start=True, stop=True)
                    gcol = rp.tile([P, 1], f32, tag="gcol")
                    nc.scalar.activation(gcol[:], gps[:], AF.Sigmoid)
                    # v = ve2 * sigmoid(g) + v_proj (ve2 pre-scaled by 2)
                    nc.vector.scalar_tensor_tensor(
                        out=vsb[:, tt, :], in0=ve2_sb[:, tt, :], scalar=gcol[:],
                        in1=tp[:], op0=ALU.mult, op1=ALU.add,
                    )

            def qk_tail(head, sl, ps):
                # rope: qr = ps*cc + swap(ps)*ss  (ss carries the sign).
                # engine split: DVE does 3 ops, Pool 1, ACT 1 — DVE was the
                # projection-phase bottleneck when it carried all of rope.
                qr = rp.tile([P, 512], f32, tag="qr")
                nc.vector.tensor_mul(qr[:], ps[:], cc_sb[:, sl])
                qs = rp.tile([P, 512], f32, tag="qs")
                nc.vector.tensor_mul(qs[0:64, :], ps[64:128, :], ss_sb[0:64, sl])
                nc.vector.tensor_mul(qs[64:128, :], ps[0:64, :], ss_sb[64:128, sl])
                nc.vector.tensor_add(qr[:], qr[:], qs[:])
                q2 = rp.tile([P, 512], bf16, tag="q2")
                nc.scalar.square(q2[:], qr[:])
                # ssq with ones[128,128] stationary: every partition gets the
                # column sum -> rms row already broadcast, no extra matmul.
                ssq = ppA.tile([P, 512], f32, tag="sc", name=f"ssq{head}_{sl.start}")
                nc.tensor.matmul(ssq[:], lhsT=ones_sb[:], rhs=q2[:],
                                 start=True, stop=True)
                srow = rm.tile([P, 512], f32, tag="srow")
                if head == 0:
                    # k: fold the 1/sqrt(d) score scale into k-hat:
                    # 1/sqrt(ssq + HD*eps) = isq/sqrt(ms + eps)
                    nc.scalar.activation(srow[:], ssq[:], AF.Sqrt,
                                         bias=epsk_sb[:], scale=1.0)
                else:
                    nc.scalar.activation(srow[:], ssq[:], AF.Sqrt,
                                         bias=epsq_sb[:], scale=1.0 / HD)
                rr = rm.tile([P, 512], f32, tag="rr")
                nc.vector.reciprocal_approx_fast(rr[:], srow[:])
                dest = khat[:, sl] if head == 0 else qhat[:, head - 2, sl]
                nc.gpsimd.tensor_mul(dest, qr[:], rr[:])

            def wave_tails(items):
                for (head, sl, ps) in items:
                    if head == 1:
                        v_tail(sl, ps)
                    else:
                        qk_tail(head, sl, ps)

            prev_items = None
            for wave in WAVES:
                items = wave_mms(wave)
                if prev_items:
                    wave_tails(prev_items)
                prev_items = items
            wave_tails(prev_items)

            # wave5 tails emitted above run on DVE/Pool/ACT while the PE
            # enters attention (qi 0..7 depend only on waves 1-4).

            # ---- attention + out-proj ---------------------------------
            # S^T tiles [kj, (h,q)] fused across the 4 heads (512 wide).
            CO = C_ // 512
            ISQ_NONE = None  # isq folded into khat

            yps = {}
            accA = {}
            accB = {}
            accb = {}
            denp = {}
            rds = {}
            yqs = {}

            def ktc_of(qi):
                return min(WT + 1, NT - qi)

            def emit_scores(qi, kk):
                kt = qi + kk
                qs4 = qhat[:, :, qi * P:(qi + 1) * P]
                sp = ppA.tile([P, GH * P], f32, tag="sc", name=f"sp{qi}_{kk}")
                nc.tensor.matmul(sp[:], lhsT=khat[:, kt * P:(kt + 1) * P],
                                 rhs=qs4, start=True, stop=True)
                pt = aw.tile([P, GH * P], bf16, tag="pt")
                nc.scalar.activation(pt[:], sp[:], AF.Exp)
                masked_lo = (kk == 0)
                masked_hi = (kk == WT and ktc_of(qi) == WT + 1)
                if masked_lo or masked_hi:
                    ptm = aw.tile([P, GH * P], bf16, tag="pt")
                    nc.vector.tensor_mul(
                        ptm[:], pt[:], mlo_sb[:] if masked_lo else mhi_sb[:])
                    pt = ptm
                return pt

            def emit_pv(qi, kk, pt):
                ktc = ktc_of(qi)
                kt = qi + kk
                if kk == 0:
                    yps[qi] = ppY.tile([P, GH * P], f32, tag="yp", name=f"yp{qi}")
                    accA[qi] = pt   # bf16 partial-sum chain heads
                elif kk == 1:
                    accB[qi] = pt
                elif kk % 2 == 0:
                    if kk == 2:
                        t_ = acp.tile([P, GH * P], bf16, tag="accA")
                        nc.vector.tensor_add(t_[:], accA[qi][:], pt[:])
                        accA[qi] = t_
                    else:
                        nc.vector.tensor_add(accA[qi][:], accA[qi][:], pt[:])
                else:
                    if kk == 3:
                        t_ = acp.tile([P, GH * P], bf16, tag="accB")
                        nc.gpsimd.tensor_add(t_[:], accB[qi][:], pt[:])
                        accB[qi] = t_
                    else:
                        nc.gpsimd.tensor_add(accB[qi][:], accB[qi][:], pt[:])
                nc.tensor.matmul(
                    yps[qi][:], lhsT=vsb[:, kt, :], rhs=pt[:],
                    start=(kk == 0), stop=(kk == ktc - 1),
                )

            def emit_merge(qi):
                if qi not in accB:
                    accb[qi] = accA[qi]
                else:
                    t_ = acp.tile([P, GH * P], bf16, tag="accM")
                    nc.vector.tensor_add(t_[:], accA[qi][:], accB[qi][:])
                    accb[qi] = t_

            def emit_den(qi):
                # one matmul: sums over kj AND broadcasts across partitions
                denp[qi] = ppD.tile([P, GH * P], f32, tag="dn", name=f"dn{qi}")
                nc.tensor.matmul(denp[qi][:], lhsT=ones_sb[:], rhs=accb[qi][:],
                                 start=True, stop=True)
                rd = rm.tile([P, GH * P], f32, tag="rd")
                nc.vector.reciprocal_approx_fast(rd[:], denp[qi][:])
                rds[qi] = rd

            def emit_yq(qi):
                yq = yqp.tile([P, GH * P], bf16, tag="yq")
                nc.vector.tensor_mul(yq[:], yps[qi][:], rds[qi][:])
                yqs[qi] = yq

            def emit_out(qi, half):
                # one half = 2 adjacent psO banks: 8 matmuls (h-outer so
                # LDWEIGHTS dedups), one paired [P,1024] DVE copy, one DMA
                qsl = slice(qi * P, (qi + 1) * P)
                yq = yqs[qi]
                o0 = ppO.tile([P, 512], f32, tag="op", name=f"op{qi}_{half}a")
                o1 = ppO.tile([P, 512], f32, tag="op", name=f"op{qi}_{half}b")
                for h in range(GH):
                    for co, ops_ in ((2 * half, o0), (2 * half + 1, o1)):
                        nc.tensor.matmul(
                            ops_[:], lhsT=yq[:, h * P:(h + 1) * P],
                            rhs=wo_sb[:, h, co * 512:(co + 1) * 512],
                            start=(h == 0), stop=(h == GH - 1),
                        )
                ob = op_pool.tile([P, 1024], bf16, tag="ob")
                nc.vector.tensor_copy(out=ob[:, 0:512], in_=o0[:])
                nc.scalar.copy(out=ob[:, 512:1024], in_=o1[:])
                nc.sync.dma_start(
                    out=out_d[qsl, half * 1024:(half + 1) * 1024], in_=ob[:])

            # main interleaved loop: scores run 4 ahead of PV (gives the
            # exp 4 PE-steps of slack); the deferred den/yq/out stages of
            # qi run spread through qi+1's stream so the PE never waits on
            # the DVE/ACT chains.
            PV_DELAY = 4

            def finish_pv(q_, k_, p_):
                emit_pv(q_, k_, p_)
                if k_ == ktc_of(q_) - 1:
                    emit_merge(q_)
                    deferred.append(lambda q=q_: emit_den(q))
                    deferred.append(lambda q=q_: emit_yq(q))
                    deferred.append(lambda q=q_: emit_out(q, 0))
                    deferred.append(lambda q=q_: emit_out(q, 1))

            pv_queue = deque()
            deferred = deque()
            for qi in range(NT):
                ktc = ktc_of(qi)
                for kk in range(ktc):
                    pt = emit_scores(qi, kk)
                    if len(pv_queue) >= PV_DELAY:
                        finish_pv(*pv_queue.popleft())
                    pv_queue.append((qi, kk, pt))
                    if deferred and kk >= 2:
                        deferred.popleft()()
            while pv_queue:
                finish_pv(*pv_queue.popleft())
            while deferred:
                deferred.popleft()()

    if dedup:
        n = dedup_ldweights(nc)
        import logging
        logging.getLogger(__name__).info(f"dedup_ldweights removed {n}")
    return nc


def _get_program(T_=T, C_=C, win=WINDOW, dedup=True):
    key = (T_, C_, win, dedup)
    if key not in _PROGRAM_CACHE:
        nc = build_program(T_, C_, win, dedup=dedup)
        nc.finalize()
        _PROGRAM_CACHE[key] = nc
    return _PROGRAM_CACHE[key]


def make_in_maps(x, ve, cos, sin, Wq, Wk, Wv, Wg, Wo):
    """Build the 8 per-core input dicts (host-side sharding/layout prep)."""
    cosT = np.ascontiguousarray(cos[:, 0, :].T).astype(np.float32)  # [64, T]
    sinT = np.ascontiguousarray(sin[:, 0, :].T).astype(np.float32)
    cc = np.concatenate([cosT, cosT], axis=0)            # [128, T]
    ss = np.concatenate([sinT, -sinT], axis=0)           # [128, T]
    # multiplicative 0/1 band masks in S^T coords [kj, q], tiled per head.
    # diag tile (kk==0): keep kj >= q ; far tile (kk==WT): keep kj < q
    kj = np.arange(P)[:, None]
    q = np.arange(P)[None, :]
    m_lo = (kj >= q).astype(np.float32)
    m_hi = (kj < q).astype(np.float32)
    mlo = np.tile(m_lo, (1, GH)).astype(BF16)
    mhi = np.tile(m_hi, (1, GH)).astype(BF16)
    identb = np.eye(P, dtype=np.float32).astype(BF16)

    in_maps = []
    for core in range(N_CORES):
        b, g = divmod(core, N_KV)
        in_maps.append({
            "xT": np.ascontiguousarray(x[b].T).astype(BF16),
            "wq": Wq[:, g * GH * HD:(g + 1) * GH * HD].astype(BF16),
            "wk": Wk[:, g * HD:(g + 1) * HD].astype(BF16),
            "wv": Wv[:, g * HD:(g + 1) * HD].astype(BF16),
            "wg": np.ascontiguousarray(Wg[:, g:g + 1]).astype(BF16),
            "ve2": (2.0 * ve[b][:, g * HD:(g + 1) * HD]).astype(BF16),
            "wo": Wo[g * GH * HD:(g + 1) * GH * HD, :].astype(BF16),
            "cc": cc.astype(BF16), "ss": ss.astype(BF16),
            "mlo": mlo, "mhi": mhi, "identb": identb,
        })
    return in_maps


def kernel(x, ve, cos, sin, Wq, Wk, Wv, Wg, Wo, window):
    assert int(window) == WINDOW and x.shape == (B, T, C)
    from concourse.bass_utils import run_bass_kernel_spmd

    nc = _get_program()
    in_maps = make_in_maps(x, ve, cos, sin, Wq, Wk, Wv, Wg, Wo)
    res = run_bass_kernel_spmd(nc, in_maps, core_ids=list(range(N_CORES)))
    out = np.zeros((B, T, C), dtype=np.float32)
    for core in range(N_CORES):
        b = core // N_KV
        out[b] += res.results[core]["out"].astype(np.float32)
    return out


# revision 26
# speedup vs baseline: 1.3761x; 1.0794x over previous
"""Trainium2 Bass kernel for a GQA sliding-window attention layer.

Reference computation (B=2, T=2048, C=2048, 16 Q heads / 4 KV heads, d=128):
    q = x @ Wq; k = x @ Wk; v = x @ Wv (+ sigmoid-gated value embedding)
    q, k = rmsnorm(rope(q)), rmsnorm(rope(k))
    scores masked to the band 0 <= j - i < window (=1024), softmax over j
    out = (p @ v) @ Wo

Sharding: 8 cores = 2 batches x 4 KV groups.  Each core computes its 4 Q
heads / 1 KV head for one batch and a partial output (its 512-row slice of
the Wo contraction); the host sums the 4 partials per batch (bf16 partials).

v2 changes vs the 377us baseline (all aimed at the PE bottleneck):
  - softmax denominators no longer use 108 ones-matmuls: exp tiles are
    accumulated on DVE/Pool (bf16 chains), then ONE ones[128,128] matmul
    per q-tile both sums over kj and broadcasts 1/den across partitions.
  - band masks are multiplicative 0/1 masks applied on Pool after exp
    (removes 24 PE bias-matmuls).
  - rms row broadcasts are free: the sum-of-squares matmul uses a
    ones[128,128] stationary, so every partition already has the row sum
    (removes 36 broadcast matmuls).
  - projections emitted in waves that reuse one stationary for 2-4
    matmuls; out-proj is h-outer/co-inner; a post-schedule IR pass
    deletes redundant InstLdweights (same weights AP, no sync info).
  - output written in bf16 (halves the 16MB/core output DMA).
"""

import numpy as np
import ml_dtypes
from collections import deque

BF16 = ml_dtypes.bfloat16

B, T, C = 2, 2048, 2048
N_HEAD, N_KV, HD, GATE_CH = 16, 4, 128, 32
WINDOW = 1024
P = 128
GH = N_HEAD // N_KV  # q heads per kv head (= per core)
N_CORES = 8

_PROGRAM_CACHE = {}


def dedup_ldweights(nc):
    """Remove InstLdweights that reload the stationary already in the PE.

    Safe subset only: the candidate must have identical (memref, offset,
    pattern, dtype) to the previous PE weight load in the same block, and
    carry no semaphore waits/updates (sync_info None).  Content safety:
    between two identical loads with no other InstLdweights in between, no
    producer rewrites the weights region (weight tiles are write-once per
    buffer rotation, and any rotation reuse has many other loads between).
    """

    def sig(inst):
        ap = inst.ins[0]
        try:
            return (ap.memref, ap.offset, str(ap.ap), str(ap.dtype))
        except Exception:
            return None

    removed = 0
    for f in nc.m.functions:
        for b in f.blocks:
            il = b.instructions
            last = None
            doomed = []
            for inst in il:
                tn = type(inst).__name__
                if tn == "InstLdweights":
                    s = sig(inst)
                    if s is not None and s == last and inst.sync_info is None:
                        doomed.append(inst)
                    else:
                        last = s
            for inst in doomed:
                il.remove(inst)
                removed += 1
    return removed


def build_program(T_=T, C_=C, win=WINDOW, dedup=True):
    import concourse.mybir as mybir
    import concourse.tile as tile
    from concourse import bacc

    dt = mybir.dt
    f32 = dt.float32
    bf16 = dt.bfloat16
    AF = mybir.ActivationFunctionType
    ALU = mybir.AluOpType

    NT = T_ // P          # token tiles
    KT = C_ // P          # contraction tiles
    WT = win // P         # window tiles
    TS = T_ // 512        # 512-wide token slices

    nc = bacc.Bacc()

    xT = nc.declare_dram_parameter("xT", [C_, T_], bf16, isOutput=False)
    wq = nc.declare_dram_parameter("wq", [C_, GH * HD], bf16, isOutput=False)
    wk = nc.declare_dram_parameter("wk", [C_, HD], bf16, isOutput=False)
    wv = nc.declare_dram_parameter("wv", [C_, HD], bf16, isOutput=False)
    wg = nc.declare_dram_parameter("wg", [GATE_CH, 1], bf16, isOutput=False)
    ve2 = nc.declare_dram_parameter("ve2", [T_, HD], bf16, isOutput=False)
    wo = nc.declare_dram_parameter("wo", [GH * HD, C_], bf16, isOutput=False)
    ccd = nc.declare_dram_parameter("cc", [P, T_], bf16, isOutput=False)
    ssd = nc.declare_dram_parameter("ss", [P, T_], bf16, isOutput=False)
    mlo = nc.declare_dram_parameter("mlo", [P, GH * P], bf16, isOutput=False)
    mhi = nc.declare_dram_parameter("mhi", [P, GH * P], bf16, isOutput=False)
    idb = nc.declare_dram_parameter("identb", [P, P], bf16, isOutput=False)
    out_d = nc.declare_dram_parameter("out", [T_, C_], bf16, isOutput=True)

    with tile.TileContext(nc) as tc:
        with (
            tc.tile_pool(name="singles", bufs=1) as sg,
            tc.tile_pool(name="rope", bufs=3) as rp,
            tc.tile_pool(name="rms", bufs=2) as rm,
            tc.tile_pool(name="attw", bufs=7) as aw,
            tc.tile_pool(name="accs", bufs=2) as acp,
            tc.tile_pool(name="yqp", bufs=2) as yqp,
            tc.tile_pool(name="outp", bufs=4) as op_pool,
            tc.tile_pool(name="psA", bufs=3, space="PSUM") as ppA,   # wavesA / scores (3)
            tc.tile_pool(name="psY", bufs=3, space="PSUM") as ppY,   # wavesB / yps+den (3)
            tc.tile_pool(name="psO", bufs=2, space="PSUM") as ppO,   # tails / out pairs (2)
        ):
            # ---- input DMAs, priority order ----------------------------
            # few, large DMAs (per-descriptor cost is ~0.4-0.6us on the
            # queue): tiny consts, whole wk/wv, then the 16 xT tiles that
            # wave1 chases, then cc/ss (first k-tail needs them ~30us in),
            # wq in 4 chunks (wave2 chases), ve2, wo (needed last).
            wg_sb = sg.tile([GATE_CH, 1], bf16, tag="wg")
            nc.sync.dma_start(out=wg_sb[:], in_=wg[:])
            mlo_sb = sg.tile([P, GH * P], bf16, tag="mlo")
            nc.sync.dma_start(out=mlo_sb[:], in_=mlo[:])
            mhi_sb = sg.tile([P, GH * P], bf16, tag="mhi")
            nc.sync.dma_start(out=mhi_sb[:], in_=mhi[:])
            idb_sb = sg.tile([P, P], bf16, tag="idb")
            nc.sync.dma_start(out=idb_sb[:], in_=idb[:])

            wq_sb = sg.tile([P, KT, GH * HD], bf16, tag="wq")
            wk_sb = sg.tile([P, KT, HD], bf16, tag="wk")
            wv_sb = sg.tile([P, KT, HD], bf16, tag="wv")
            wqr = wq.rearrange("(o p) n -> p o n", p=P)
            nc.sync.dma_start(out=wk_sb[:], in_=wk.rearrange("(o p) n -> p o n", p=P))
            nc.sync.dma_start(out=wv_sb[:], in_=wv.rearrange("(o p) n -> p o n", p=P))
            xt = []
            for kt in range(KT):
                t_ = sg.tile([P, T_], bf16, tag=f"xt{kt}")
                nc.sync.dma_start(out=t_[:], in_=xT[kt * P:(kt + 1) * P, :])
                xt.append(t_)
            cc_sb = sg.tile([P, T_], bf16, tag="cc")
            nc.sync.dma_start(out=cc_sb[:], in_=ccd[:])
            ss_sb = sg.tile([P, T_], bf16, tag="ss")
            nc.sync.dma_start(out=ss_sb[:], in_=ssd[:])
            for qc in range(4):
                nc.sync.dma_start(out=wq_sb[:, 4 * qc:4 * (qc + 1), :],
                                  in_=wqr[:, 4 * qc:4 * (qc + 1), :])
            ve2_sb = sg.tile([P, NT, HD], bf16, tag="ve2")
            nc.sync.dma_start(out=ve2_sb[:], in_=ve2.rearrange("(o p) d -> p o d", p=P))
            wo_sb = sg.tile([P, GH, C_], bf16, tag="wo")
            nc.sync.dma_start(out=wo_sb[:], in_=wo.rearrange("(o p) n -> p o n", p=P))

            ones_sb = sg.tile([P, P], bf16, tag="ones128")
            nc.vector.memset(ones_sb[:], 1.0)
            epsq_sb = sg.tile([P, 1], f32, tag="epsq")
            nc.vector.memset(epsq_sb[:], 1e-6)
            epsk_sb = sg.tile([P, 1], f32, tag="epsk")
            nc.vector.memset(epsk_sb[:], HD * 1e-6)

            # persistent intermediates
            qhat = sg.tile([P, GH, T_], bf16, tag="qhat")   # normalized roped q, [d, h, t]
            khat = sg.tile([P, T_], bf16, tag="khat")       # normalized roped k * isq
            vsb = sg.tile([P, NT, HD], bf16, tag="vsb")     # gated v, [tok, tt, d]

            # ---- projections: double-buffered waves of 3 ---------------
            # head ids: 0=k, 1=v, 2..5=q0..q3.  Waves alternate between
            # two 3-bank psum pools, so a new wave's kt=0 matmuls never
            # wait for the previous wave's tails to free banks.  After
            # wave6's tails attention qi=0..7 is unlocked; waves 7-8 keep
            # the PE fed while those tails run.
            WAVES = [
                [(0, 0), (0, 1), (0, 2)],
                [(0, 3), (1, 0), (1, 1)],
                [(1, 2), (1, 3), (2, 0)],
                [(2, 1), (3, 0), (3, 1)],
                [(4, 0), (4, 1), (5, 0)],
                [(5, 1), (2, 2), (2, 3)],
                [(3, 2), (3, 3), (4, 2)],
                [(4, 3), (5, 2), (5, 3)],
            ]

            def w_ap(head, kt):
                if head == 0:
                    return wk_sb[:, kt, :]
                if head == 1:
                    return wv_sb[:, kt, :]
                return wq_sb[:, kt, (head - 2) * HD:(head - 1) * HD]

            def wave_mms(wi, wave):
                pool_, tag_ = (ppA, "sc") if wi % 2 == 0 else (ppY, "yp")
                items = []
                for (head, ts_) in wave:
                    sl = slice(ts_ * 512, ts_ * 512 + 512)
                    ps = pool_.tile([P, 512], f32, tag=tag_, name=f"ps{head}_{ts_}")
                    items.append((head, sl, ps))
                for kt in range(KT):
                    # group by head so consecutive matmuls share one stationary
                    for gi, (head, ts_) in enumerate(wave):
                        nc.tensor.matmul(
                            items[gi][2][:], lhsT=w_ap(head, kt),
                            rhs=xt[kt][:, items[gi][1]],
                            start=(kt == 0), stop=(kt == KT - 1),
                        )
                return items

            def v_tail(sl, ps):
                # vT psum [d, tok] -> bf16 sbuf, PE-transpose each 128-tok
                # block to [tok, d], add sigmoid-gated ve.
                vt = rp.tile([P, 512], bf16, tag="vt")
                nc.scalar.copy(out=vt[:], in_=ps[:])
                for i in range(4):
                    tt = sl.start // P + i
                    tsl = slice(tt * P, (tt + 1) * P)
                    tp = ppO.tile([P, P], bf16, tag="op", name=f"tp{tt}")
                    nc.tensor.transpose(tp[:], vt[:, i * P:(i + 1) * P], idb_sb[:])
                    gps = ppO.tile([P, 1], f32, tag="op", name=f"gp{tt}")
                    nc.tensor.matmul(gps[:], lhsT=xt[0][0:GATE_CH, tsl],
                                     rhs=wg_sb[:], start=True, stop=True)
                    gcol = rp.tile([P, 1], f32, tag="gcol")
                    nc.scalar.activation(gcol[:], gps[:], AF.Sigmoid)
                    # v = ve2 * sigmoid(g) + v_proj (ve2 pre-scaled by 2)
                    nc.vector.scalar_tensor_tensor(
                        out=vsb[:, tt, :], in0=ve2_sb[:, tt, :], scalar=gcol[:],
                        in1=tp[:], op0=ALU.mult, op1=ALU.add,
                    )

            def qk_tail(head, sl, ps):
                # rope: qr = ps*cc + swap(ps)*ss  (ss carries the sign).
                # engine split: DVE does 3 ops, Pool 1, ACT 1 — DVE was the
                # projection-phase bottleneck when it carried all of rope.
                qr = rp.tile([P, 512], f32, tag="qr")
                nc.vector.tensor_mul(qr[:], ps[:], cc_sb[:, sl])
                qs = rp.tile([P, 512], f32, tag="qs")
                nc.vector.tensor_mul(qs[0:64, :], ps[64:128, :], ss_sb[0:64, sl])
                nc.vector.tensor_mul(qs[64:128, :], ps[0:64, :], ss_sb[64:128, sl])
                nc.vector.tensor_add(qr[:], qr[:], qs[:])
                q2 = rp.tile([P, 512], bf16, tag="q2")
                nc.gpsimd.tensor_mul(q2[:], qr[:], qr[:])
                # ssq with ones[128,128] stationary: every partition gets the
                # column sum -> rms row already broadcast, no extra matmul.
                ssq = ppO.tile([P, 512], f32, tag="op", name=f"ssq{head}_{sl.start}")
                nc.tensor.matmul(ssq[:], lhsT=ones_sb[:], rhs=q2[:],
                                 start=True, stop=True)
                srow = rm.tile([P, 512], f32, tag="srow")
                if head == 0:
                    # k: fold the 1/sqrt(d) score scale into k-hat:
                    # 1/sqrt(ssq + HD*eps) = isq/sqrt(ms + eps)
                    nc.scalar.activation(srow[:], ssq[:], AF.Sqrt,
                                         bias=epsk_sb[:], scale=1.0)
                else:
                    nc.scalar.activation(srow[:], ssq[:], AF.Sqrt,
                                         bias=epsq_sb[:], scale=1.0 / HD)
                rr = rm.tile([P, 512], f32, tag="rr")
                nc.vector.reciprocal_approx_fast(rr[:], srow[:])
                dest = khat[:, sl] if head == 0 else qhat[:, head - 2, sl]
                nc.gpsimd.tensor_mul(dest, qr[:], rr[:])

            def wave_tails(items):
                for (head, sl, ps) in items:
                    if head == 1:
                        v_tail(sl, ps)
                    else:
                        qk_tail(head, sl, ps)

            prev_items = None
            for wi, wave in enumerate(WAVES):
                items = wave_mms(wi, wave)
                if prev_items:
                    wave_tails(prev_items)
                prev_items = items
            wave_tails(prev_items)

            # wave5 tails emitted above run on DVE/Pool/ACT while the PE
            # enters attention (qi 0..7 depend only on waves 1-4).

            # ---- attention + out-proj ---------------------------------
            # S^T tiles [kj, (h,q)] fused across the 4 heads (512 wide).
            CO = C_ // 512
            ISQ_NONE = None  # isq folded into khat

            yps = {}
            accA = {}
            accB = {}
            accb = {}
            denp = {}
            rds = {}
            yqs = {}

            def ktc_of(qi):
                return min(WT + 1, NT - qi)

            def emit_scores(qi, kk):
                kt = qi + kk
                qs4 = qhat[:, :, qi * P:(qi + 1) * P]
                sp = ppA.tile([P, GH * P], f32, tag="sc", name=f"sp{qi}_{kk}")
                nc.tensor.matmul(sp[:], lhsT=khat[:, kt * P:(kt + 1) * P],
                                 rhs=qs4, start=True, stop=True)
                pt = aw.tile([P, GH * P], bf16, tag="pt")
                nc.scalar.activation(pt[:], sp[:], AF.Exp)
                masked_lo = (kk == 0)
                masked_hi = (kk == WT and ktc_of(qi) == WT + 1)
                if masked_lo or masked_hi:
                    ptm = aw.tile([P, GH * P], bf16, tag="pt")
                    nc.vector.tensor_mul(
                        ptm[:], pt[:], mlo_sb[:] if masked_lo else mhi_sb[:])
                    pt = ptm
                return pt

            def emit_pv(qi, kk, pt):
                ktc = ktc_of(qi)
                kt = qi + kk
                if kk == 0:
                    yps[qi] = ppY.tile([P, GH * P], f32, tag="yp", name=f"yp{qi}")
                    accA[qi] = pt   # bf16 partial-sum chain heads
                elif kk == 1:
                    accB[qi] = pt
                elif kk % 2 == 0:
                    if kk == 2:
                        t_ = acp.tile([P, GH * P], bf16, tag="accA")
                        nc.vector.tensor_add(t_[:], accA[qi][:], pt[:])
                        accA[qi] = t_
                    else:
                        nc.vector.tensor_add(accA[qi][:], accA[qi][:], pt[:])
                else:
                    if kk == 3:
                        t_ = acp.tile([P, GH * P], bf16, tag="accB")
                        nc.gpsimd.tensor_add(t_[:], accB[qi][:], pt[:])
                        accB[qi] = t_
                    else:
                        nc.gpsimd.tensor_add(accB[qi][:], accB[qi][:], pt[:])
                nc.tensor.matmul(
                    yps[qi][:], lhsT=vsb[:, kt, :], rhs=pt[:],
                    start=(kk == 0), stop=(kk == ktc - 1),
                )

            def emit_merge(qi):
                if qi not in accB:
                    accb[qi] = accA[qi]
                else:
                    t_ = acp.tile([P, GH * P], bf16, tag="accM")
                    nc.vector.tensor_add(t_[:], accA[qi][:], accB[qi][:])
                    accb[qi] = t_

            def emit_den(qi):
                # one matmul: sums over kj AND broadcasts across partitions
                denp[qi] = ppY.tile([P, GH * P], f32, tag="yp", name=f"dn{qi}")
                nc.tensor.matmul(denp[qi][:], lhsT=ones_sb[:], rhs=accb[qi][:],
                                 start=True, stop=True)
                rd = rm.tile([P, GH * P], f32, tag="rd")
                nc.vector.reciprocal_approx_fast(rd[:], denp[qi][:])
                rds[qi] = rd

            def emit_yq(qi):
                yq = yqp.tile([P, GH * P], bf16, tag="yq")
                nc.vector.tensor_mul(yq[:], yps[qi][:], rds[qi][:])
                yqs[qi] = yq

            def emit_out(qi, half):
                # one half = 2 adjacent psO banks: 8 matmuls (h-outer so
                # LDWEIGHTS dedups), one paired [P,1024] DVE copy, one DMA
                qsl = slice(qi * P, (qi + 1) * P)
                yq = yqs[qi]
                o0 = ppO.tile([P, 512], f32, tag="op", name=f"op{qi}_{half}a")
                o1 = ppO.tile([P, 512], f32, tag="op", name=f"op{qi}_{half}b")
                for h in range(GH):
                    for co, ops_ in ((2 * half, o0), (2 * half + 1, o1)):
                        nc.tensor.matmul(
                            ops_[:], lhsT=yq[:, h * P:(h + 1) * P],
                            rhs=wo_sb[:, h, co * 512:(co + 1) * 512],
                            start=(h == 0), stop=(h == GH - 1),
                        )
                ob = op_pool.tile([P, 1024], bf16, tag="ob")
                nc.vector.tensor_copy(out=ob[:, 0:512], in_=o0[:])
                nc.scalar.copy(out=ob[:, 512:1024], in_=o1[:])
                nc.sync.dma_start(
                    out=out_d[qsl, half * 1024:(half + 1) * 1024], in_=ob[:])

            # main interleaved loop: scores run 4 ahead of PV (gives the
            # exp 4 PE-steps of slack); the deferred den/yq/out stages of
            # qi run spread through qi+1's stream so the PE never waits on
            # the DVE/ACT chains.
            PV_DELAY = 4

            def finish_pv(q_, k_, p_):
                emit_pv(q_, k_, p_)
                if k_ == ktc_of(q_) - 1:
                    emit_merge(q_)
                    deferred.append(lambda q=q_: emit_den(q))
                    deferred.append(lambda q=q_: emit_yq(q))
                    deferred.append(lambda q=q_: emit_out(q, 0))
                    deferred.append(lambda q=q_: emit_out(q, 1))

            pv_queue = deque()
            deferred = deque()
            for qi in range(NT):
                ktc = ktc_of(qi)
                for kk in range(ktc):
                    pt = emit_scores(qi, kk)
                    # invariant: at most one qi's stages (4) may be pending
                    # when a PV is emitted — its psum-ring slots need the
                    # den/yq consumers of qi-2 emitted first
                    while len(deferred) > 4:
                        deferred.popleft()()
                    if len(pv_queue) >= PV_DELAY:
                        finish_pv(*pv_queue.popleft())
                    pv_queue.append((qi, kk, pt))
                    if deferred and kk >= 2:
                        deferred.popleft()()
            while pv_queue:
                while len(deferred) > 4:
                    deferred.popleft()()
                finish_pv(*pv_queue.popleft())
            while deferred:
                deferred.popleft()()

    if dedup:
        n = dedup_ldweights(nc)
        import logging
        logging.getLogger(__name__).info(f"dedup_ldweights removed {n}")
    return nc


def _get_program(T_=T, C_=C, win=WINDOW, dedup=True):
    key = (T_, C_, win, dedup)
    if key not in _PROGRAM_CACHE:
        nc = build_program(T_, C_, win, dedup=dedup)
        nc.finalize()
        _PROGRAM_CACHE[key] = nc
    return _PROGRAM_CACHE[key]


def make_in_maps(x, ve, cos, sin, Wq, Wk, Wv, Wg, Wo):
    """Build the 8 per-core input dicts (host-side sharding/layout prep)."""
    cosT = np.ascontiguousarray(cos[:, 0, :].T).astype(np.float32)  # [64, T]
    sinT = np.ascontiguousarray(sin[:, 0, :].T).astype(np.float32)
    cc = np.concatenate([cosT, cosT], axis=0)            # [128, T]
    ss = np.concatenate([sinT, -sinT], axis=0)           # [128, T]
    # multiplicative 0/1 band masks in S^T coords [kj, q], tiled per head.
    # diag tile (kk==0): keep kj >= q ; far tile (kk==WT): keep kj < q
    kj = np.arange(P)[:, None]
    q = np.arange(P)[None, :]
    m_lo = (kj >= q).astype(np.float32)
    m_hi = (kj < q).astype(np.float32)
    mlo = np.tile(m_lo, (1, GH)).astype(BF16)
    mhi = np.tile(m_hi, (1, GH)).astype(BF16)
    identb = np.eye(P, dtype=np.float32).astype(BF16)

    in_maps = []
    for core in range(N_CORES):
        b, g = divmod(core, N_KV)
        in_maps.append({
            "xT": np.ascontiguousarray(x[b].T).astype(BF16),
            "wq": Wq[:, g * GH * HD:(g + 1) * GH * HD].astype(BF16),
            "wk": Wk[:, g * HD:(g + 1) * HD].astype(BF16),
            "wv": Wv[:, g * HD:(g + 1) * HD].astype(BF16),
            "wg": np.ascontiguousarray(Wg[:, g:g + 1]).astype(BF16),
            "ve2": (2.0 * ve[b][:, g * HD:(g + 1) * HD]).astype(BF16),
            "wo": Wo[g * GH * HD:(g + 1) * GH * HD, :].astype(BF16),
            "cc": cc.astype(BF16), "ss": ss.astype(BF16),
            "mlo": mlo, "mhi": mhi, "identb": identb,
        })
    return in_maps


def kernel(x, ve, cos, sin, Wq, Wk, Wv, Wg, Wo, window):
    assert int(window) == WINDOW and x.shape == (B, T, C)
    from concourse.bass_utils import run_bass_kernel_spmd

    nc = _get_program()
    in_maps = make_in_maps(x, ve, cos, sin, Wq, Wk, Wv, Wg, Wo)
    res = run_bass_kernel_spmd(nc, in_maps, core_ids=list(range(N_CORES)))
    out = np.zeros((B, T, C), dtype=np.float32)
    for core in range(N_CORES):
        b = core // N_KV
        out[b] += res.results[core]["out"].astype(np.float32)
    return out


# revision 31
# speedup vs baseline: 1.4198x; 1.0317x over previous
"""Trainium2 Bass kernel for a GQA sliding-window attention layer.

Reference computation (B=2, T=2048, C=2048, 16 Q heads / 4 KV heads, d=128):
    q = x @ Wq; k = x @ Wk; v = x @ Wv (+ sigmoid-gated value embedding)
    q, k = rmsnorm(rope(q)), rmsnorm(rope(k))
    scores masked to the band 0 <= j - i < window (=1024), softmax over j
    out = (p @ v) @ Wo

Sharding: 8 cores = 2 batches x 4 KV groups.  Each core computes its 4 Q
heads / 1 KV head for one batch and a partial output (its 512-row slice of
the Wo contraction); the host sums the 4 partials per batch (bf16 partials).

v2 changes vs the 377us baseline (all aimed at the PE bottleneck):
  - softmax denominators no longer use 108 ones-matmuls: exp tiles are
    accumulated on DVE/Pool (bf16 chains), then ONE ones[128,128] matmul
    per q-tile both sums over kj and broadcasts 1/den across partitions.
  - band masks are multiplicative 0/1 masks applied on Pool after exp
    (removes 24 PE bias-matmuls).
  - rms row broadcasts are free: the sum-of-squares matmul uses a
    ones[128,128] stationary, so every partition already has the row sum
    (removes 36 broadcast matmuls).
  - projections emitted in waves that reuse one stationary for 2-4
    matmuls; out-proj is h-outer/co-inner; a post-schedule IR pass
    deletes redundant InstLdweights (same weights AP, no sync info).
  - output written in bf16 (halves the 16MB/core output DMA).
"""

import numpy as np
import ml_dtypes
from collections import deque

BF16 = ml_dtypes.bfloat16

B, T, C = 2, 2048, 2048
N_HEAD, N_KV, HD, GATE_CH = 16, 4, 128, 32
WINDOW = 1024
P = 128
GH = N_HEAD // N_KV  # q heads per kv head (= per core)
N_CORES = 8

_PROGRAM_CACHE = {}


def dedup_ldweights(nc):
    """Remove InstLdweights that reload the stationary already in the PE.

    Safe subset only: the candidate must have identical (memref, offset,
    pattern, dtype) to the previous PE weight load in the same block, and
    carry no semaphore waits/updates (sync_info None).  Content safety:
    between two identical loads with no other InstLdweights in between, no
    producer rewrites the weights region (weight tiles are write-once per
    buffer rotation, and any rotation reuse has many other loads between).
    """

    def sig(inst):
        ap = inst.ins[0]
        try:
            return (ap.memref, ap.offset, str(ap.ap), str(ap.dtype))
        except Exception:
            return None

    removed = 0
    for f in nc.m.functions:
        for b in f.blocks:
            il = b.instructions
            last = None
            doomed = []
            for inst in il:
                tn = type(inst).__name__
                if tn == "InstLdweights":
                    s = sig(inst)
                    if s is not None and s == last and inst.sync_info is None:
                        doomed.append(inst)
                    else:
                        last = s
            for inst in doomed:
                il.remove(inst)
                removed += 1
    return removed


def build_program(T_=T, C_=C, win=WINDOW, dedup=True):
    import concourse.mybir as mybir
    import concourse.tile as tile
    from concourse import bacc

    dt = mybir.dt
    f32 = dt.float32
    bf16 = dt.bfloat16
    AF = mybir.ActivationFunctionType
    ALU = mybir.AluOpType

    NT = T_ // P          # token tiles
    KT = C_ // P          # contraction tiles
    WT = win // P         # window tiles
    TS = T_ // 512        # 512-wide token slices

    nc = bacc.Bacc()

    xT = nc.declare_dram_parameter("xT", [C_, T_], bf16, isOutput=False)
    wq = nc.declare_dram_parameter("wq", [C_, GH * HD], bf16, isOutput=False)
    wk = nc.declare_dram_parameter("wk", [C_, HD], bf16, isOutput=False)
    wv = nc.declare_dram_parameter("wv", [C_, HD], bf16, isOutput=False)
    wg = nc.declare_dram_parameter("wg", [GATE_CH, 1], bf16, isOutput=False)
    ve2 = nc.declare_dram_parameter("ve2", [T_, HD], bf16, isOutput=False)
    wo = nc.declare_dram_parameter("wo", [GH * HD, C_], bf16, isOutput=False)
    ccd = nc.declare_dram_parameter("cc", [P, T_], bf16, isOutput=False)
    ssd = nc.declare_dram_parameter("ss", [P, T_], bf16, isOutput=False)
    mlo = nc.declare_dram_parameter("mlo", [P, GH * P], bf16, isOutput=False)
    mhi = nc.declare_dram_parameter("mhi", [P, GH * P], bf16, isOutput=False)
    idb = nc.declare_dram_parameter("identb", [P, P], bf16, isOutput=False)
    out_d = nc.declare_dram_parameter("out", [T_, C_], bf16, isOutput=True)

    with tile.TileContext(nc) as tc:
        with (
            tc.tile_pool(name="singles", bufs=1) as sg,
            tc.tile_pool(name="rope", bufs=3) as rp,
            tc.tile_pool(name="rms", bufs=2) as rm,
            tc.tile_pool(name="attw", bufs=7) as aw,
            tc.tile_pool(name="accs", bufs=2) as acp,
            tc.tile_pool(name="yqp", bufs=2) as yqp,
            tc.tile_pool(name="outp", bufs=4) as op_pool,
            tc.tile_pool(name="psA", bufs=3, space="PSUM") as ppA,   # wavesA / scores (3)
            tc.tile_pool(name="psY", bufs=3, space="PSUM") as ppY,   # wavesB / yps+den (3)
            tc.tile_pool(name="psO", bufs=2, space="PSUM") as ppO,   # tails / out pairs (2)
        ):
            # ---- input DMAs, priority order ----------------------------
            # few, large DMAs (per-descriptor cost is ~0.4-0.6us on the
            # queue): tiny consts, whole wk/wv, then the 16 xT tiles that
            # wave1 chases, then cc/ss (first k-tail needs them ~30us in),
            # wq in 4 chunks (wave2 chases), ve2, wo (needed last).
            wq_sb = sg.tile([P, KT, GH * HD], bf16, tag="wq")
            wk_sb = sg.tile([P, KT, HD], bf16, tag="wk")
            wv_sb = sg.tile([P, KT, HD], bf16, tag="wv")
            wqr = wq.rearrange("(o p) n -> p o n", p=P)
            wkr = wk.rearrange("(o p) n -> p o n", p=P)
            # wk in 4 chunks so wave1's first matmuls start immediately
            for qc in range(4):
                nc.sync.dma_start(out=wk_sb[:, 4 * qc:4 * (qc + 1), :],
                                  in_=wkr[:, 4 * qc:4 * (qc + 1), :])
            nc.sync.dma_start(out=wv_sb[:], in_=wv.rearrange("(o p) n -> p o n", p=P))
            xt = []
            for kt in range(KT):
                t_ = sg.tile([P, T_], bf16, tag=f"xt{kt}")
                nc.sync.dma_start(out=t_[:], in_=xT[kt * P:(kt + 1) * P, :])
                xt.append(t_)
            cc_sb = sg.tile([P, T_], bf16, tag="cc")
            ss_sb = sg.tile([P, T_], bf16, tag="ss")
            for qc in range(4):
                nc.sync.dma_start(out=wq_sb[:, 4 * qc:4 * (qc + 1), :],
                                  in_=wqr[:, 4 * qc:4 * (qc + 1), :])
                if qc == 1:
                    # rope tables land before the first k-tails need them,
                    # without delaying the xT stream wave1 chases
                    nc.sync.dma_start(out=cc_sb[:], in_=ccd[:])
                    nc.sync.dma_start(out=ss_sb[:], in_=ssd[:])
            wg_sb = sg.tile([GATE_CH, 1], bf16, tag="wg")
            nc.sync.dma_start(out=wg_sb[:], in_=wg[:])
            mlo_sb = sg.tile([P, GH * P], bf16, tag="mlo")
            nc.sync.dma_start(out=mlo_sb[:], in_=mlo[:])
            mhi_sb = sg.tile([P, GH * P], bf16, tag="mhi")
            nc.sync.dma_start(out=mhi_sb[:], in_=mhi[:])
            idb_sb = sg.tile([P, P], bf16, tag="idb")
            nc.sync.dma_start(out=idb_sb[:], in_=idb[:])
            ve2_sb = sg.tile([P, NT, HD], bf16, tag="ve2")
            nc.sync.dma_start(out=ve2_sb[:], in_=ve2.rearrange("(o p) d -> p o d", p=P))
            wo_sb = sg.tile([P, GH, C_], bf16, tag="wo")
            nc.sync.dma_start(out=wo_sb[:], in_=wo.rearrange("(o p) n -> p o n", p=P))

            ones_sb = sg.tile([P, P], bf16, tag="ones128")
            nc.vector.memset(ones_sb[:], 1.0)
            epsq_sb = sg.tile([P, 1], f32, tag="epsq")
            nc.vector.memset(epsq_sb[:], 1e-6)
            epsk_sb = sg.tile([P, 1], f32, tag="epsk")
            nc.vector.memset(epsk_sb[:], HD * 1e-6)

            # persistent intermediates
            qhat = sg.tile([P, GH, T_], bf16, tag="qhat")   # normalized roped q, [d, h, t]
            khat = sg.tile([P, T_], bf16, tag="khat")       # normalized roped k * isq
            vsb = sg.tile([P, NT, HD], bf16, tag="vsb")     # gated v, [tok, tt, d]

            # ---- projections: double-buffered waves of 3 ---------------
            # head ids: 0=k, 1=v, 2..5=q0..q3.  Waves alternate between
            # two 3-bank psum pools, so a new wave's kt=0 matmuls never
            # wait for the previous wave's tails to free banks.  After
            # wave6's tails attention qi=0..7 is unlocked; waves 7-8 keep
            # the PE fed while those tails run.
            WAVES = [
                [(0, 0), (0, 1), (0, 2)],
                [(0, 3), (1, 0), (1, 1)],
                [(1, 2), (1, 3), (2, 0)],
                [(2, 1), (3, 0), (3, 1)],
                [(4, 0), (4, 1), (5, 0)],
                [(5, 1), (2, 2), (2, 3)],
                [(3, 2), (3, 3), (4, 2)],
                [(4, 3), (5, 2), (5, 3)],
            ]

            def w_ap(head, kt):
                if head == 0:
                    return wk_sb[:, kt, :]
                if head == 1:
                    return wv_sb[:, kt, :]
                return wq_sb[:, kt, (head - 2) * HD:(head - 1) * HD]

            def wave_mms(wi, wave):
                pool_, tag_ = (ppA, "sc") if wi % 2 == 0 else (ppY, "yp")
                items = []
                for (head, ts_) in wave:
                    sl = slice(ts_ * 512, ts_ * 512 + 512)
                    ps = pool_.tile([P, 512], f32, tag=tag_, name=f"ps{head}_{ts_}")
                    items.append((head, sl, ps))
                for kt in range(KT):
                    # group by head so consecutive matmuls share one stationary
                    for gi, (head, ts_) in enumerate(wave):
                        nc.tensor.matmul(
                            items[gi][2][:], lhsT=w_ap(head, kt),
                            rhs=xt[kt][:, items[gi][1]],
                            start=(kt == 0), stop=(kt == KT - 1),
                        )
                return items

            def v_tail(sl, ps):
                # vT psum [d, tok] -> bf16 sbuf, PE-transpose each 128-tok
                # block to [tok, d], add sigmoid-gated ve.
                vt = rp.tile([P, 512], bf16, tag="vt")
                nc.scalar.copy(out=vt[:], in_=ps[:])
                for i in range(4):
                    tt = sl.start // P + i
                    tsl = slice(tt * P, (tt + 1) * P)
                    tp = ppO.tile([P, P], bf16, tag="op", name=f"tp{tt}")
                    nc.tensor.transpose(tp[:], vt[:, i * P:(i + 1) * P], idb_sb[:])
                    gps = ppO.tile([P, 1], f32, tag="op", name=f"gp{tt}")
                    nc.tensor.matmul(gps[:], lhsT=xt[0][0:GATE_CH, tsl],
                                     rhs=wg_sb[:], start=True, stop=True)
                    gcol = rp.tile([P, 1], f32, tag="gcol")
                    nc.scalar.activation(gcol[:], gps[:], AF.Sigmoid)
                    # v = ve2 * sigmoid(g) + v_proj (ve2 pre-scaled by 2)
                    nc.vector.scalar_tensor_tensor(
                        out=vsb[:, tt, :], in0=ve2_sb[:, tt, :], scalar=gcol[:],
                        in1=tp[:], op0=ALU.mult, op1=ALU.add,
                    )

            def qk_tail(head, sl, ps):
                # rope: qr = ps*cc + swap(ps)*ss  (ss carries the sign).
                # engine split: DVE does 3 ops, Pool 1, ACT 1 — DVE was the
                # projection-phase bottleneck when it carried all of rope.
                qr = rp.tile([P, 512], f32, tag="qr")
                nc.vector.tensor_mul(qr[:], ps[:], cc_sb[:, sl])
                qs = rp.tile([P, 512], f32, tag="qs")
                nc.vector.tensor_mul(qs[0:64, :], ps[64:128, :], ss_sb[0:64, sl])
                nc.vector.tensor_mul(qs[64:128, :], ps[0:64, :], ss_sb[64:128, sl])
                nc.vector.tensor_add(qr[:], qr[:], qs[:])
                q2 = rp.tile([P, 512], bf16, tag="q2")
                nc.gpsimd.tensor_mul(q2[:], qr[:], qr[:])
                # ssq with ones[128,128] stationary: every partition gets the
                # column sum -> rms row already broadcast, no extra matmul.
                ssq = ppO.tile([P, 512], f32, tag="op", name=f"ssq{head}_{sl.start}")
                nc.tensor.matmul(ssq[:], lhsT=ones_sb[:], rhs=q2[:],
                                 start=True, stop=True)
                srow = rm.tile([P, 512], f32, tag="srow")
                if head == 0:
                    # k: fold the 1/sqrt(d) score scale into k-hat:
                    # 1/sqrt(ssq + HD*eps) = isq/sqrt(ms + eps)
                    nc.scalar.activation(srow[:], ssq[:], AF.Sqrt,
                                         bias=epsk_sb[:], scale=1.0)
                else:
                    nc.scalar.activation(srow[:], ssq[:], AF.Sqrt,
                                         bias=epsq_sb[:], scale=1.0 / HD)
                rr = rm.tile([P, 512], f32, tag="rr")
                nc.vector.reciprocal_approx_fast(rr[:], srow[:])
                dest = khat[:, sl] if head == 0 else qhat[:, head - 2, sl]
                nc.gpsimd.tensor_mul(dest, qr[:], rr[:])

            def wave_tails(items):
                for (head, sl, ps) in items:
                    if head == 1:
                        v_tail(sl, ps)
                    else:
                        qk_tail(head, sl, ps)

            prev_items = None
            for wi, wave in enumerate(WAVES):
                items = wave_mms(wi, wave)
                if prev_items:
                    wave_tails(prev_items)
                prev_items = items
            wave_tails(prev_items)

            # wave5 tails emitted above run on DVE/Pool/ACT while the PE
            # enters attention (qi 0..7 depend only on waves 1-4).

            # ---- attention + out-proj ---------------------------------
            # S^T tiles [kj, (h,q)] fused across the 4 heads (512 wide).
            CO = C_ // 512
            ISQ_NONE = None  # isq folded into khat

            yps = {}
            accA = {}
            accB = {}
            accb = {}
            denp = {}
            rds = {}
            yqs = {}

            def ktc_of(qi):
                return min(WT + 1, NT - qi)

            def emit_scores(qi, kk):
                kt = qi + kk
                qs4 = qhat[:, :, qi * P:(qi + 1) * P]
                sp = ppA.tile([P, GH * P], f32, tag="sc", name=f"sp{qi}_{kk}")
                nc.tensor.matmul(sp[:], lhsT=khat[:, kt * P:(kt + 1) * P],
                                 rhs=qs4, start=True, stop=True)
                pt = aw.tile([P, GH * P], bf16, tag="pt")
                nc.scalar.activation(pt[:], sp[:], AF.Exp)
                masked_lo = (kk == 0)
                masked_hi = (kk == WT and ktc_of(qi) == WT + 1)
                if masked_lo or masked_hi:
                    ptm = aw.tile([P, GH * P], bf16, tag="pt")
                    nc.vector.tensor_mul(
                        ptm[:], pt[:], mlo_sb[:] if masked_lo else mhi_sb[:])
                    pt = ptm
                return pt

            # first few q-tiles compute den on the PE (ones-matmul per kk):
            # at attention start DVE/Pool are still draining the last
            # projection tails, so the bf16 chains would stall the den.
            PE_DEN = frozenset(range(3))

            def emit_pv(qi, kk, pt):
                ktc = ktc_of(qi)
                kt = qi + kk
                if kk == 0:
                    yps[qi] = ppY.tile([P, GH * P], f32, tag="yp", name=f"yp{qi}")
                if qi in PE_DEN:
                    if kk == 0:
                        denp[qi] = ppY.tile([P, GH * P], f32, tag="yp",
                                            name=f"dn{qi}")
                    nc.tensor.matmul(
                        denp[qi][:], lhsT=ones_sb[:], rhs=pt[:],
                        start=(kk == 0), stop=(kk == ktc - 1),
                    )
                elif kk == 0:
                    accA[qi] = pt   # bf16 partial-sum chain heads
                elif kk == 1:
                    accB[qi] = pt
                elif kk % 2 == 0:
                    if kk == 2:
                        t_ = acp.tile([P, GH * P], bf16, tag="accA")
                        nc.vector.tensor_add(t_[:], accA[qi][:], pt[:])
                        accA[qi] = t_
                    else:
                        nc.vector.tensor_add(accA[qi][:], accA[qi][:], pt[:])
                else:
                    if kk == 3:
                        t_ = acp.tile([P, GH * P], bf16, tag="accB")
                        nc.gpsimd.tensor_add(t_[:], accB[qi][:], pt[:])
                        accB[qi] = t_
                    else:
                        nc.gpsimd.tensor_add(accB[qi][:], accB[qi][:], pt[:])
                nc.tensor.matmul(
                    yps[qi][:], lhsT=vsb[:, kt, :], rhs=pt[:],
                    start=(kk == 0), stop=(kk == ktc - 1),
                )

            def emit_merge(qi):
                if qi in PE_DEN:
                    return
                if qi not in accB:
                    accb[qi] = accA[qi]
                else:
                    t_ = acp.tile([P, GH * P], bf16, tag="accM")
                    nc.vector.tensor_add(t_[:], accA[qi][:], accB[qi][:])
                    accb[qi] = t_

            def emit_den_yq(qi):
                if qi not in PE_DEN:
                    # one matmul: sums over kj AND broadcasts across partitions
                    denp[qi] = ppY.tile([P, GH * P], f32, tag="yp",
                                        name=f"dn{qi}")
                    nc.tensor.matmul(denp[qi][:], lhsT=ones_sb[:],
                                     rhs=accb[qi][:], start=True, stop=True)
                rd = rm.tile([P, GH * P], f32, tag="rd")
                nc.vector.reciprocal_approx_fast(rd[:], denp[qi][:])
                rds[qi] = rd
                yq = yqp.tile([P, GH * P], bf16, tag="yq")
                nc.vector.tensor_mul(yq[:], yps[qi][:], rds[qi][:])
                yqs[qi] = yq

            def emit_out(qi, half):
                # one half = 2 adjacent psO banks: 8 matmuls (h-outer so
                # LDWEIGHTS dedups), one paired [P,1024] DVE copy, one DMA
                qsl = slice(qi * P, (qi + 1) * P)
                yq = yqs[qi]
                o0 = ppO.tile([P, 512], f32, tag="op", name=f"op{qi}_{half}a")
                o1 = ppO.tile([P, 512], f32, tag="op", name=f"op{qi}_{half}b")
                for h in range(GH):
                    for co, ops_ in ((2 * half, o0), (2 * half + 1, o1)):
                        nc.tensor.matmul(
                            ops_[:], lhsT=yq[:, h * P:(h + 1) * P],
                            rhs=wo_sb[:, h, co * 512:(co + 1) * 512],
                            start=(h == 0), stop=(h == GH - 1),
                        )
                ob = op_pool.tile([P, 1024], bf16, tag="ob")
                nc.vector.tensor_copy(out=ob[:, 0:512], in_=o0[:])
                nc.scalar.copy(out=ob[:, 512:1024], in_=o1[:])
                nc.sync.dma_start(
                    out=out_d[qsl, half * 1024:(half + 1) * 1024], in_=ob[:])

            # main interleaved loop: scores run 4 ahead of PV (gives the
            # exp 4 PE-steps of slack); the deferred den/yq/out stages of
            # qi run spread through qi+1's stream so the PE never waits on
            # the DVE/ACT chains.
            PV_DELAY = 4

            def finish_pv(q_, k_, p_):
                emit_pv(q_, k_, p_)
                if k_ == ktc_of(q_) - 1:
                    emit_merge(q_)
                    deferred.append(lambda q=q_: emit_den_yq(q))
                    deferred.append(lambda q=q_: emit_out(q, 0))
                    deferred.append(lambda q=q_: emit_out(q, 1))

            pv_queue = deque()
            deferred = deque()
            for qi in range(NT):
                ktc = ktc_of(qi)
                for kk in range(ktc):
                    pt = emit_scores(qi, kk)
                    # invariant: at most one qi's stages (3) may be pending
                    # when a PV is emitted — its psum-ring slots need the
                    # den/yq consumers of qi-2 emitted first
                    while len(deferred) > 3:
                        deferred.popleft()()
                    if len(pv_queue) >= PV_DELAY:
                        finish_pv(*pv_queue.popleft())
                    pv_queue.append((qi, kk, pt))
                    if deferred and kk >= 2:
                        deferred.popleft()()
            while pv_queue:
                while len(deferred) > 3:
                    deferred.popleft()()
                finish_pv(*pv_queue.popleft())
            while deferred:
                deferred.popleft()()

    if dedup:
        n = dedup_ldweights(nc)
        import logging
        logging.getLogger(__name__).info(f"dedup_ldweights removed {n}")
    return nc


def _get_program(T_=T, C_=C, win=WINDOW, dedup=True):
    key = (T_, C_, win, dedup)
    if key not in _PROGRAM_CACHE:
        nc = build_program(T_, C_, win, dedup=dedup)
        nc.finalize()
        _PROGRAM_CACHE[key] = nc
    return _PROGRAM_CACHE[key]


def make_in_maps(x, ve, cos, sin, Wq, Wk, Wv, Wg, Wo):
    """Build the 8 per-core input dicts (host-side sharding/layout prep)."""
    cosT = np.ascontiguousarray(cos[:, 0, :].T).astype(np.float32)  # [64, T]
    sinT = np.ascontiguousarray(sin[:, 0, :].T).astype(np.float32)
    cc = np.concatenate([cosT, cosT], axis=0)            # [128, T]
    ss = np.concatenate([sinT, -sinT], axis=0)           # [128, T]
    # multiplicative 0/1 band masks in S^T coords [kj, q], tiled per head.
    # diag tile (kk==0): keep kj >= q ; far tile (kk==WT): keep kj < q
    kj = np.arange(P)[:, None]
    q = np.arange(P)[None, :]
    m_lo = (kj >= q).astype(np.float32)
    m_hi = (kj < q).astype(np.float32)
    mlo = np.tile(m_lo, (1, GH)).astype(BF16)
    mhi = np.tile(m_hi, (1, GH)).astype(BF16)
    identb = np.eye(P, dtype=np.float32).astype(BF16)

    in_maps = []
    for core in range(N_CORES):
        b, g = divmod(core, N_KV)
        in_maps.append({
            "xT": np.ascontiguousarray(x[b].T).astype(BF16),
            "wq": Wq[:, g * GH * HD:(g + 1) * GH * HD].astype(BF16),
            "wk": Wk[:, g * HD:(g + 1) * HD].astype(BF16),
            "wv": Wv[:, g * HD:(g + 1) * HD].astype(BF16),
            "wg": np.ascontiguousarray(Wg[:, g:g + 1]).astype(BF16),
            "ve2": (2.0 * ve[b][:, g * HD:(g + 1) * HD]).astype(BF16),
            "wo": Wo[g * GH * HD:(g + 1) * GH * HD, :].astype(BF16),
            "cc": cc.astype(BF16), "ss": ss.astype(BF16),
            "mlo": mlo, "mhi": mhi, "identb": identb,
        })
    return in_maps


def kernel(x, ve, cos, sin, Wq, Wk, Wv, Wg, Wo, window):
    assert int(window) == WINDOW and x.shape == (B, T, C)
    from concourse.bass_utils import run_bass_kernel_spmd

    nc = _get_program()
    in_maps = make_in_maps(x, ve, cos, sin, Wq, Wk, Wv, Wg, Wo)
    res = run_bass_kernel_spmd(nc, in_maps, core_ids=list(range(N_CORES)))
    out = np.zeros((B, T, C), dtype=np.float32)
    for core in range(N_CORES):
        b = core // N_KV
        out[b] += res.results[core]["out"].astype(np.float32)
    return out
